# revision 7
# baseline (speedup 1.0000x reference)
"""DGLFRM forward pass as a distributed Bass kernel on 8 TRN2 NeuronCores.

Sharding: nodes row-sharded across 8 cores (1024 rows each). adj_mat is
symmetric, so each core loads the COLUMN slice (adj+I)[:, rows_c] in bf16,
which is exactly the transposed-lhs layout the TensorEngine needs for
S = (adj+I) @ Y restricted to its rows. Degrees: free-dim reduce of the
column slice gives per-core partials of every row's degree (AllReduce-summed),
and a ones-vector matmul against the resident slice gives own-row degrees
without any core-dependent indexing (the program is pure SPMD). Activations
are AllGathered in bf16 between the two propagation rounds; the z->h2
latent is transposed on-device and AllGathered for the h2 @ h2.T decode.
"""
import sys
if '/opt/trn_rl_repo' not in sys.path:
    sys.path.insert(0, '/opt/trn_rl_repo')

import numpy as np
import ml_dtypes

import concourse.bass as bass
import concourse.bacc as bacc
import concourse.tile as tile
from concourse import mybir
from concourse.bass_utils import run_bass_kernel_spmd

BF = mybir.dt.bfloat16
F32 = mybir.dt.float32
EPS = 1e-7
P = 128


class Cfg:
    def __init__(self, N=8192, D=1024, H=256, K=64, HD=32, NC=8):
        self.N, self.D, self.H, self.K, self.HD, self.NC = N, D, H, K, HD, NC
        self.R = N // NC          # rows per core
        self.NKB = N // P         # k-blocks over full node dim
        self.MT = self.R // P     # m-tiles per core
        self.DKB = D // P         # k-blocks over feature dim


def _chunks(total, step):
    out = []
    o = 0
    while o < total:
        out.append((o, min(step, total - o)))
        o += step
    return out


def build_bass(cfg: Cfg):
    N, D, H, K, HD, NC = cfg.N, cfg.D, cfg.H, cfg.K, cfg.HD, cfg.NC
    R, NKB, MT, DKB = cfg.R, cfg.NKB, cfg.MT, cfg.DKB
    K3 = 3 * K
    MG = min(MT, 4)               # m-tiles per accumulation group
    rg = [list(range(NC))]

    nc = bacc.Bacc("TRN2", target_bir_lowering=False, debug=False, num_devices=NC)

    # ---- I/O ----
    a_cols = nc.declare_dram_parameter("a_cols", [N, R], BF, isOutput=False)
    xT = nc.declare_dram_parameter("xT", [D, R], BF, isOutput=False)
    w1 = nc.declare_dram_parameter("w1", [D, H], BF, isOutput=False)
    b1 = nc.declare_dram_parameter("b1", [1, H], F32, isOutput=False)
    wall = nc.declare_dram_parameter("wall", [H, K3], BF, isOutput=False)
    ball = nc.declare_dram_parameter("ball", [1, K3], F32, isOutput=False)
    wd = nc.declare_dram_parameter("wd", [K, HD], BF, isOutput=False)
    bd = nc.declare_dram_parameter("bd", [1, HD], F32, isOutput=False)
    wx = nc.declare_dram_parameter("wx", [K, D], BF, isOutput=False)
    bx = nc.declare_dram_parameter("bx", [1, D], F32, isOutput=False)
    beta_a = nc.declare_dram_parameter("beta_a", [1, K], F32, isOutput=False)
    beta_b = nc.declare_dram_parameter("beta_b", [1, K], F32, isOutput=False)
    u_beta = nc.declare_dram_parameter("u_beta", [R, K], F32, isOutput=False)
    u_bern = nc.declare_dram_parameter("u_bern", [R, K], F32, isOutput=False)
    eps_r = nc.declare_dram_parameter("eps_r", [R, K], F32, isOutput=False)
    tri = nc.declare_dram_parameter("tri", [K, K], F32, isOutput=False)
    ident = nc.declare_dram_parameter("ident", [P, P], F32, isOutput=False)
    xhat_out = nc.declare_dram_parameter("xhat", [R, D], F32, isOutput=True)
    edge_out = nc.declare_dram_parameter("edge", [R, N], F32, isOutput=True)

    AG, AR = "AllGather", "AllReduce"
    BYP, ADD = mybir.AluOpType.bypass, mybir.AluOpType.add
    MUL, SUB = mybir.AluOpType.mult, mybir.AluOpType.subtract
    AF = mybir.ActivationFunctionType

    with tile.TileContext(nc) as tc:
        with tc.tile_pool(name="abig", bufs=NKB) as apool, \
             tc.tile_pool(name="xtp", bufs=3) as xpool, \
             tc.tile_pool(name="const", bufs=1) as cpool, \
             tc.tile_pool(name="gath", bufs=3) as gpool, \
             tc.tile_pool(name="evict", bufs=3) as epool, \
             tc.tile_pool(name="work", bufs=2) as wpool, \
             tc.tile_pool(name="stage", bufs=4) as spool, \
             tc.tile_pool(name="psum", bufs=5, space="PSUM") as pp, \
             tc.tile_pool(name="psumd", bufs=2, space="PSUM") as ppd, \
             tc.tile_pool(name="psums", bufs=1, space="PSUM") as pps, \
             tc.tile_pool(name="dram", bufs=1, space="DRAM") as dram:

            # ---------- constants ----------
            identity = cpool.tile([P, P], F32)
            nc.sync.dma_start(out=identity[:], in_=ident[:])
            tri_t = cpool.tile([K, K], F32)
            nc.sync.dma_start(out=tri_t[:], in_=tri[:])
            w1_t = cpool.tile([P, DKB * H], BF)
            for kd in range(DKB):
                nc.sync.dma_start(out=w1_t[:, kd * H:(kd + 1) * H],
                                  in_=w1[kd * P:(kd + 1) * P, :])
            wall_t = cpool.tile([P, (H // P) * K3], BF)
            for hcv in range(H // P):
                nc.sync.dma_start(out=wall_t[:, hcv * K3:(hcv + 1) * K3],
                                  in_=wall[hcv * P:(hcv + 1) * P, :])
            wd_t = cpool.tile([K, HD], BF)
            nc.sync.dma_start(out=wd_t[:], in_=wd[:])
            wx_t = cpool.tile([K, D], BF)
            nc.sync.dma_start(out=wx_t[:], in_=wx[:])
            # biases broadcast to all partitions via DMA
            b1_t = cpool.tile([P, H], F32)
            nc.sync.dma_start(out=b1_t[:], in_=b1.ap().to_broadcast([P, H]))
            ball_t = cpool.tile([P, K3], F32)
            nc.sync.dma_start(out=ball_t[:], in_=ball.ap().to_broadcast([P, K3]))
            bd_t = cpool.tile([P, HD], F32)
            nc.sync.dma_start(out=bd_t[:], in_=bd.ap().to_broadcast([P, HD]))
            bx_t = cpool.tile([P, D], F32)
            nc.sync.dma_start(out=bx_t[:], in_=bx.ap().to_broadcast([P, D]))
            # inv_a, inv_b computed redundantly on every partition
            ba_t = cpool.tile([P, K], F32)
            nc.sync.dma_start(out=ba_t[:], in_=beta_a.ap().to_broadcast([P, K]))
            bb_t = cpool.tile([P, K], F32)
            nc.sync.dma_start(out=bb_t[:], in_=beta_b.ap().to_broadcast([P, K]))
            inva = cpool.tile([P, K], F32)
            nc.scalar.activation(inva[:], ba_t[:], AF.Exp)
            nc.scalar.activation(inva[:], inva[:], AF.Ln, bias=1.0)
            nc.vector.reciprocal(inva[:], inva[:])
            invb = cpool.tile([P, K], F32)
            nc.scalar.activation(invb[:], bb_t[:], AF.Exp)
            nc.scalar.activation(invb[:], invb[:], AF.Ln, bias=1.0)
            nc.vector.reciprocal(invb[:], invb[:])
            epsb = cpool.tile([P, 1], F32)
            nc.vector.memset(epsb[:], EPS)
            ones_p1 = cpool.tile([P, 1], BF)
            nc.vector.memset(ones_p1[:], 1.0)
            ones_11 = cpool.tile([1, 1], F32)
            nc.vector.memset(ones_11[:], 1.0)

            # ------- phase 1: A load + degree partials + own degrees + xw -------
            a_res = []
            degp = cpool.tile([P, NKB], F32)        # partial degrees (permuted)
            dch = _chunks(R, 512)
            deg_own_ps = [ppd.tile([1, dw], F32, tag="dacc", name="degown")
                          for (_, dw) in dch]
            for k in range(NKB):
                at = apool.tile([P, R], BF, tag="ablk", name="ablk")
                nc.sync.dma_start(out=at[:], in_=a_cols[k * P:(k + 1) * P, :])
                a_res.append(at)
                # partial degree of global rows [kP, (k+1)P) over our columns
                nc.vector.tensor_reduce(out=degp[:, k:k + 1], in_=at[:],
                                        axis=mybir.AxisListType.X, op=ADD)
                # own-node degrees: column sums via ones-matmul
                for ci, (co, cw) in enumerate(dch):
                    nc.tensor.matmul(deg_own_ps[ci][:], ones_p1[:],
                                     at[:, co:co + cw],
                                     start=(k == 0), stop=(k == NKB - 1))

            # own-node d = 1/sqrt(deg); spread [1,R] -> [P, MT]
            dow_f = cpool.tile([1, R], F32)
            for ci, (co, cw) in enumerate(dch):
                nc.scalar.activation(dow_f[:, co:co + cw], deg_own_ps[ci][:],
                                     AF.Ln)
            nc.scalar.activation(dow_f[:], dow_f[:], AF.Exp, scale=-0.5)
            down = cpool.tile([P, MT], F32)
            for m in range(MT):
                psd = pps.tile([P, 1], F32, tag="spread", name="spread")
                nc.tensor.matmul(psd[:], dow_f[:, m * P:(m + 1) * P], ones_11[:],
                                 start=True, stop=True)
                nc.vector.tensor_copy(out=down[:, m:m + 1], in_=psd[:])

            # xw = x_c @ W1  (grouped m-outer; xT re-streamed per group)
            cc_xw_in = dram.tile([R, H], BF)
            for g0, gw in _chunks(MT, MG):
                xw_ps = [pp.tile([P, H], F32, tag="acc", name="xwacc")
                         for _ in range(gw)]
                for kd in range(DKB):
                    xt = xpool.tile([P, R], BF, tag="xblk", name="xblk")
                    nc.sync.dma_start(out=xt[:], in_=xT[kd * P:(kd + 1) * P, :])
                    for mi in range(gw):
                        m = g0 + mi
                        nc.tensor.matmul(xw_ps[mi][:], xt[:, m * P:(m + 1) * P],
                                         w1_t[:, kd * H:(kd + 1) * H],
                                         start=(kd == 0), stop=(kd == DKB - 1))
                for mi in range(gw):
                    m = g0 + mi
                    xwb = epool.tile([P, H], BF, tag="ev", name="xwb")
                    nc.vector.tensor_tensor(out=xwb[:], in0=xw_ps[mi][:],
                                            in1=b1_t[:], op=ADD)
                    nc.gpsimd.dma_start(out=cc_xw_in[m * P:(m + 1) * P, :],
                                        in_=xwb[:])

            # AllGather xw (bf16)
            cc_xw_out = dram.tile([N, H], BF)
            nc.gpsimd.collective_compute(AG, BYP, replica_groups=rg,
                                         ins=[cc_xw_in.opt()],
                                         outs=[cc_xw_out.opt()])

            # AllReduce degree partials
            cc_deg_in = dram.tile([P, NKB], F32)
            cc_deg_out = dram.tile([P, NKB], F32)
            nc.gpsimd.dma_start(out=cc_deg_in[:], in_=degp[:])
            nc.gpsimd.collective_compute(AR, ADD, replica_groups=rg,
                                         ins=[cc_deg_in.opt()],
                                         outs=[cc_deg_out.opt()])
            d_t = cpool.tile([P, NKB], F32)
            nc.sync.dma_start(out=d_t[:], in_=cc_deg_out[:])
            nc.scalar.activation(d_t[:], d_t[:], AF.Ln)
            nc.scalar.activation(d_t[:], d_t[:], AF.Exp, scale=-0.5)

            # ------- phase 2: S1 = (A+I)_cols.T @ (d * XW_full) -------
            hT = [cpool.tile([P, R], BF, tag="hT", name="hT", bufs=H // P)
                  for _ in range(H // P)]
            cc_m2_in = dram.tile([R, K3], BF)
            for g0, gw in _chunks(MT, MG):
                s1_acc = [pp.tile([P, H], F32, tag="acc", name="s1acc")
                          for _ in range(gw)]
                for k in range(NKB):
                    gblk = gpool.tile([P, H], BF, tag="gblk", name="gblk")
                    nc.sync.dma_start(out=gblk[:],
                                      in_=cc_xw_out[k * P:(k + 1) * P, :])
                    rsc = gpool.tile([P, H], BF, tag="rsc", name="rsc")
                    nc.vector.tensor_scalar(out=rsc[:], in0=gblk[:],
                                            scalar1=d_t[:, k:k + 1], scalar2=None,
                                            op0=MUL)
                    for mi in range(gw):
                        m = g0 + mi
                        nc.tensor.matmul(s1_acc[mi][:],
                                         a_res[k][:, m * P:(m + 1) * P], rsc[:],
                                         start=(k == 0), stop=(k == NKB - 1))
                # h = leaky_relu(d_own * S1); store transposed bf16; M2 local
                for mi in range(gw):
                    m = g0 + mi
                    htmp = wpool.tile([P, H], F32, tag="htmp", name="htmp")
                    nc.scalar.activation(htmp[:], s1_acc[mi][:], AF.Lrelu,
                                         scale=down[:, m:m + 1], alpha=0.01)
                    for hcv in range(H // P):
                        pst = pp.tile([P, P], F32, tag="acc", name="ptr")
                        nc.tensor.transpose(pst[:], htmp[:, hcv * P:(hcv + 1) * P],
                                            identity[:])
                        nc.vector.tensor_copy(out=hT[hcv][:, m * P:(m + 1) * P],
                                              in_=pst[:])
                    ps2 = pp.tile([P, K3], F32, tag="acc", name="m2acc")
                    for hcv in range(H // P):
                        nc.tensor.matmul(ps2[:], hT[hcv][:, m * P:(m + 1) * P],
                                         wall_t[:, hcv * K3:(hcv + 1) * K3],
                                         start=(hcv == 0),
                                         stop=(hcv == H // P - 1))
                    m2b = epool.tile([P, K3], BF, tag="ev", name="m2b")
                    nc.vector.tensor_tensor(out=m2b[:], in0=ps2[:], in1=ball_t[:],
                                            op=ADD)
                    nc.gpsimd.dma_start(out=cc_m2_in[m * P:(m + 1) * P, :],
                                        in_=m2b[:])

            cc_m2_out = dram.tile([N, K3], BF)
            nc.gpsimd.collective_compute(AG, BYP, replica_groups=rg,
                                         ins=[cc_m2_in.opt()],
                                         outs=[cc_m2_out.opt()])

            # ------- phase 3: S2 + elementwise chain + z -------
            zT = cpool.tile([K, R], BF)
            for g0, gw in _chunks(MT, MG):
                s2_acc = [pp.tile([P, K3], F32, tag="acc", name="s2acc")
                          for _ in range(gw)]
                for k in range(NKB):
                    gblk2 = gpool.tile([P, K3], BF, tag="gblk", name="gblk2")
                    nc.sync.dma_start(out=gblk2[:],
                                      in_=cc_m2_out[k * P:(k + 1) * P, :])
                    rsc2 = gpool.tile([P, K3], BF, tag="rsc", name="rsc2")
                    nc.vector.tensor_scalar(out=rsc2[:], in0=gblk2[:],
                                            scalar1=d_t[:, k:k + 1], scalar2=None,
                                            op0=MUL)
                    for mi in range(gw):
                        m = g0 + mi
                        nc.tensor.matmul(s2_acc[mi][:],
                                         a_res[k][:, m * P:(m + 1) * P], rsc2[:],
                                         start=(k == 0), stop=(k == NKB - 1))
                for mi in range(gw):
                    m = g0 + mi
                    s2f = wpool.tile([P, K3], F32, tag="s2f", name="s2f")
                    nc.vector.tensor_scalar(out=s2f[:], in0=s2_acc[mi][:],
                                            scalar1=down[:, m:m + 1], scalar2=None,
                                            op0=MUL)
                    ub = wpool.tile([P, K], F32, tag="u", name="ub", bufs=3)
                    nc.sync.dma_start(out=ub[:], in_=u_beta[m * P:(m + 1) * P, :])
                    un = wpool.tile([P, K], F32, tag="u2", name="un", bufs=3)
                    nc.sync.dma_start(out=un[:], in_=u_bern[m * P:(m + 1) * P, :])
                    er = wpool.tile([P, K], F32, tag="u3", name="er", bufs=3)
                    nc.sync.dma_start(out=er[:], in_=eps_r[m * P:(m + 1) * P, :])

                    # v = (1 - u^(1/b))^(1/a);  LV = ln(v + EPS)
                    e1 = wpool.tile([P, K], F32, tag="e", name="e1", bufs=8)
                    nc.scalar.activation(e1[:], ub[:], AF.Ln)
                    nc.vector.tensor_tensor(out=e1[:], in0=e1[:], in1=invb[:],
                                            op=MUL)
                    nc.scalar.activation(e1[:], e1[:], AF.Exp)
                    nc.vector.tensor_scalar(out=e1[:], in0=e1[:], scalar1=-1.0,
                                            scalar2=1.0, op0=MUL, op1=ADD)
                    nc.scalar.activation(e1[:], e1[:], AF.Ln)
                    nc.vector.tensor_tensor(out=e1[:], in0=e1[:], in1=inva[:],
                                            op=MUL)
                    nc.scalar.activation(e1[:], e1[:], AF.Exp)
                    nc.scalar.activation(e1[:], e1[:], AF.Ln, bias=epsb[:])
                    # cumsum along K via transpose + tri matmul
                    pst = pp.tile([P, P], F32, tag="acc", name="ptr2")
                    nc.tensor.transpose(pst[:K, :], e1[:], identity[:])
                    lvt = wpool.tile([K, P], F32, tag="lvt", name="lvt")
                    nc.vector.tensor_copy(out=lvt[:], in_=pst[:K, :])
                    psc = pp.tile([P, K], F32, tag="acc", name="csacc")
                    nc.tensor.matmul(psc[:], lvt[:], tri_t[:], start=True,
                                     stop=True)
                    # prior logit
                    pe = wpool.tile([P, K], F32, tag="e", name="pe", bufs=8)
                    nc.scalar.activation(pe[:], psc[:], AF.Exp)
                    pr = wpool.tile([P, K], F32, tag="e", name="pr", bufs=8)
                    nc.scalar.activation(pr[:], pe[:], AF.Ln, bias=epsb[:])
                    t2 = wpool.tile([P, K], F32, tag="e", name="t2", bufs=8)
                    nc.vector.tensor_scalar(out=t2[:], in0=pe[:], scalar1=-1.0,
                                            scalar2=1.0 + EPS, op0=MUL, op1=ADD)
                    nc.scalar.activation(t2[:], t2[:], AF.Ln)
                    nc.vector.tensor_tensor(out=pr[:], in0=pr[:], in1=t2[:],
                                            op=SUB)
                    # y = pi_logit + prior + ln(u+EPS) - ln(1-u+EPS)
                    lb1 = wpool.tile([P, K], F32, tag="e", name="lb1", bufs=8)
                    nc.scalar.activation(lb1[:], un[:], AF.Ln, bias=epsb[:])
                    nc.vector.tensor_scalar(out=un[:], in0=un[:], scalar1=-1.0,
                                            scalar2=1.0 + EPS, op0=MUL, op1=ADD)
                    nc.scalar.activation(un[:], un[:], AF.Ln)
                    yv = wpool.tile([P, K], F32, tag="e", name="yv", bufs=8)
                    nc.vector.tensor_tensor(out=yv[:], in0=s2f[:, 0:K], in1=pr[:],
                                            op=ADD)
                    nc.vector.tensor_tensor(out=yv[:], in0=yv[:], in1=lb1[:],
                                            op=ADD)
                    nc.vector.tensor_tensor(out=yv[:], in0=yv[:], in1=un[:],
                                            op=SUB)
                    nc.scalar.activation(yv[:], yv[:], AF.Exp, scale=-1.0)
                    nc.vector.tensor_scalar(out=yv[:], in0=yv[:], scalar1=1.0,
                                            scalar2=None, op0=ADD)
                    nc.vector.reciprocal(yv[:], yv[:])
                    # r = r_mean + exp(r_log_std) * eps_r;  z = gate * r
                    ex = wpool.tile([P, K], F32, tag="e", name="ex", bufs=8)
                    nc.scalar.activation(ex[:], s2f[:, 2 * K:3 * K], AF.Exp)
                    nc.vector.tensor_tensor(out=ex[:], in0=ex[:], in1=er[:],
                                            op=MUL)
                    nc.vector.tensor_tensor(out=ex[:], in0=ex[:],
                                            in1=s2f[:, K:2 * K], op=ADD)
                    nc.vector.tensor_tensor(out=ex[:], in0=ex[:], in1=yv[:],
                                            op=MUL)
                    # transpose z -> zT
                    psz = pp.tile([P, P], F32, tag="acc", name="ztr")
                    nc.tensor.transpose(psz[:K, :], ex[:], identity[:])
                    nc.vector.tensor_copy(out=zT[:, m * P:(m + 1) * P],
                                          in_=psz[:K, :])

            # ------- phase 4: decode -------
            # h2 = z @ Wd + bd, transposed, AllGathered
            h2T = cpool.tile([HD, R], BF)
            cc_h2_in = dram.tile([HD, R], BF)
            for m in range(MT):
                psh = pp.tile([P, HD], F32, tag="acc", name="h2acc")
                nc.tensor.matmul(psh[:], zT[:, m * P:(m + 1) * P], wd_t[:],
                                 start=True, stop=True)
                h2f = wpool.tile([P, HD], F32, tag="h2f", name="h2f")
                nc.vector.tensor_tensor(out=h2f[:], in0=psh[:], in1=bd_t[:],
                                        op=ADD)
                pst2 = pp.tile([P, P], F32, tag="acc", name="h2tr")
                nc.tensor.transpose(pst2[:HD, :], h2f[:], identity[:])
                nc.vector.tensor_copy(out=h2T[:, m * P:(m + 1) * P],
                                      in_=pst2[:HD, :])
            nc.gpsimd.dma_start(out=cc_h2_in[:], in_=h2T[:])
            cc_h2_out = dram.tile([NC * HD, R], BF)
            nc.gpsimd.collective_compute(AG, BYP, replica_groups=rg,
                                         ins=[cc_h2_in.opt()],
                                         outs=[cc_h2_out.opt()])

            # xhat = z @ Wx + bx
            for m in range(MT):
                for (co, cw) in _chunks(D, 512):
                    psx = pp.tile([P, 512], F32, tag="acc", name="xhacc")
                    nc.tensor.matmul(psx[:, :cw], zT[:, m * P:(m + 1) * P],
                                     wx_t[:, co:co + cw], start=True, stop=True)
                    st = spool.tile([P, 512], F32, tag="st", name="stx")
                    nc.vector.tensor_tensor(out=st[:, :cw], in0=psx[:, :cw],
                                            in1=bx_t[:, co:co + cw], op=ADD)
                    nc.sync.dma_start(out=xhat_out[m * P:(m + 1) * P, co:co + cw],
                                      in_=st[:, :cw])

            # edge = h2 @ h2_full.T
            for nr in range(NC):
                for (co, cw) in _chunks(R, 512):
                    rblk = gpool.tile([HD, 512], BF, tag="rblk", name="rblk")
                    nc.sync.dma_start(out=rblk[:, :cw],
                                      in_=cc_h2_out[nr * HD:(nr + 1) * HD,
                                                    co:co + cw])
                    for m in range(MT):
                        pse = pp.tile([P, 512], F32, tag="acc", name="edacc")
                        nc.tensor.matmul(pse[:, :cw], h2T[:, m * P:(m + 1) * P],
                                         rblk[:, :cw], start=True, stop=True)
                        st = spool.tile([P, 512], F32, tag="st", name="ste")
                        nc.vector.tensor_copy(out=st[:, :cw], in_=pse[:, :cw])
                        nc.sync.dma_start(
                            out=edge_out[m * P:(m + 1) * P,
                                         nr * R + co:nr * R + co + cw],
                            in_=st[:, :cw])

    nc.compile()
    return nc


def make_in_maps(cfg: Cfg, inputs):
    N, D, H, K, HD, NC, R = cfg.N, cfg.D, cfg.H, cfg.K, cfg.HD, cfg.NC, cfg.R
    bf = ml_dtypes.bfloat16
    f32 = np.float32
    A = np.asarray(inputs['adj_mat'], f32)
    x = np.asarray(inputs['x'], f32)
    Abf = A.astype(bf)
    xTbf = np.ascontiguousarray(np.asarray(x.T)).astype(bf)
    shared = {
        'w1': np.asarray(inputs['W1'], f32).astype(bf),
        'b1': np.asarray(inputs['b1'], f32).reshape(1, H),
        'wall': np.concatenate([np.asarray(inputs['Wpi'], f32),
                                np.asarray(inputs['Wm'], f32),
                                np.asarray(inputs['Wls'], f32)],
                               axis=1).astype(bf),
        'ball': np.concatenate([np.asarray(inputs['bpi'], f32),
                                np.asarray(inputs['bm'], f32),
                                np.asarray(inputs['bls'], f32)]).reshape(1, 3 * K),
        'wd': np.asarray(inputs['Wd'], f32).astype(bf),
        'bd': np.asarray(inputs['bd'], f32).reshape(1, HD),
        'wx': np.asarray(inputs['Wx'], f32).astype(bf),
        'bx': np.asarray(inputs['bx'], f32).reshape(1, D),
        'beta_a': np.asarray(inputs['beta_a'], f32).reshape(1, K),
        'beta_b': np.asarray(inputs['beta_b'], f32).reshape(1, K),
        'tri': np.triu(np.ones((K, K), f32)),
        'ident': np.eye(P, dtype=f32),
    }
    u_beta = np.asarray(inputs['u_beta'], f32)
    u_bern = np.asarray(inputs['u_bern'], f32)
    eps_r = np.asarray(inputs['eps_r'], f32)
    in_maps = []
    diag = np.arange(R)
    for c in range(NC):
        c0 = c * R
        ac = np.ascontiguousarray(Abf[:, c0:c0 + R])
        ac[c0 + diag, diag] += np.asarray(1.0, bf)   # fold in +I (exact in bf16)
        in_maps.append({
            'a_cols': ac,
            'xT': np.ascontiguousarray(xTbf[:, c0:c0 + R]),
            'u_beta': np.ascontiguousarray(u_beta[c0:c0 + R]),
            'u_bern': np.ascontiguousarray(u_bern[c0:c0 + R]),
            'eps_r': np.ascontiguousarray(eps_r[c0:c0 + R]),
            **shared,
        })
    return in_maps


_CACHE = {}


def _get_nc(cfg: Cfg):
    key = (cfg.N, cfg.D, cfg.H, cfg.K, cfg.HD, cfg.NC)
    if key not in _CACHE:
        _CACHE[key] = build_bass(cfg)
    return _CACHE[key]


def run(cfg: Cfg, inputs, trace=False):
    nc = _get_nc(cfg)
    in_maps = make_in_maps(cfg, inputs)
    res = run_bass_kernel_spmd(nc, in_maps, list(range(cfg.NC)), trace=trace)
    xhat = np.concatenate([r['xhat'] for r in res.results], axis=0)
    edge = np.concatenate([r['edge'] for r in res.results], axis=0)
    return (xhat.reshape(-1).astype(np.float32),
            edge.reshape(-1).astype(np.float32)), res


def kernel(**inputs):
    cfg = Cfg()
    out, _ = run(cfg, inputs, trace=False)
    return out


# revision 8
# speedup vs baseline: 1.2269x; 1.2269x over previous
"""DGLFRM forward pass as a distributed Bass kernel on 8 TRN2 NeuronCores.

Sharding: nodes row-sharded across 8 cores (1024 rows each). adj_mat is
symmetric, so each core loads the COLUMN slice (adj+I)[:, rows_c] in bf16,
which is exactly the transposed-lhs layout the TensorEngine needs for
S = (adj+I) @ Y restricted to its rows. A ones-vector matmul against the
resident slice gives own-row degrees locally (column sums == row sums by
symmetry), so normalization is applied PRODUCER-side: each core scales the
activations it contributes by d_own before the AllGather, and row-scales its
matmul outputs by d_own afterwards -- no degree collective, no
core-dependent indexing (the program is pure SPMD). The z->h2 latent is
transposed on-device and AllGathered for the h2 @ h2.T decode.
"""
import sys
if '/opt/trn_rl_repo' not in sys.path:
    sys.path.insert(0, '/opt/trn_rl_repo')

import numpy as np
import ml_dtypes

import concourse.bass as bass
import concourse.bacc as bacc
import concourse.tile as tile
from concourse import mybir
from concourse.bass_utils import run_bass_kernel_spmd

BF = mybir.dt.bfloat16
F32 = mybir.dt.float32
EPS = 1e-7
P = 128


class Cfg:
    def __init__(self, N=8192, D=1024, H=256, K=64, HD=32, NC=8):
        self.N, self.D, self.H, self.K, self.HD, self.NC = N, D, H, K, HD, NC
        self.R = N // NC          # rows per core
        self.NKB = N // P         # k-blocks over full node dim
        self.MT = self.R // P     # m-tiles per core
        self.DKB = D // P         # k-blocks over feature dim


def _chunks(total, step):
    out = []
    o = 0
    while o < total:
        out.append((o, min(step, total - o)))
        o += step
    return out


def build_bass(cfg: Cfg):
    N, D, H, K, HD, NC = cfg.N, cfg.D, cfg.H, cfg.K, cfg.HD, cfg.NC
    R, NKB, MT, DKB = cfg.R, cfg.NKB, cfg.MT, cfg.DKB
    K3 = 3 * K
    MG = min(MT, 4)               # m-tiles per accumulation group
    rg = [list(range(NC))]

    nc = bacc.Bacc("TRN2", target_bir_lowering=False, debug=False, num_devices=NC)

    # ---- I/O ----
    a_cols = nc.declare_dram_parameter("a_cols", [N, R], BF, isOutput=False)
    xT = nc.declare_dram_parameter("xT", [D, R], BF, isOutput=False)
    w1 = nc.declare_dram_parameter("w1", [D, H], BF, isOutput=False)
    b1 = nc.declare_dram_parameter("b1", [1, H], F32, isOutput=False)
    wall = nc.declare_dram_parameter("wall", [H, K3], BF, isOutput=False)
    ball = nc.declare_dram_parameter("ball", [1, K3], F32, isOutput=False)
    wd = nc.declare_dram_parameter("wd", [K, HD], BF, isOutput=False)
    bd = nc.declare_dram_parameter("bd", [1, HD], F32, isOutput=False)
    wx = nc.declare_dram_parameter("wx", [K, D], BF, isOutput=False)
    bx = nc.declare_dram_parameter("bx", [1, D], F32, isOutput=False)
    beta_a = nc.declare_dram_parameter("beta_a", [1, K], F32, isOutput=False)
    beta_b = nc.declare_dram_parameter("beta_b", [1, K], F32, isOutput=False)
    u_beta = nc.declare_dram_parameter("u_beta", [R, K], F32, isOutput=False)
    u_bern = nc.declare_dram_parameter("u_bern", [R, K], F32, isOutput=False)
    eps_r = nc.declare_dram_parameter("eps_r", [R, K], F32, isOutput=False)
    tri = nc.declare_dram_parameter("tri", [K, K], F32, isOutput=False)
    ident = nc.declare_dram_parameter("ident", [P, P], F32, isOutput=False)
    xhat_out = nc.declare_dram_parameter("xhat", [R, D], F32, isOutput=True)
    edge_out = nc.declare_dram_parameter("edge", [R, N], F32, isOutput=True)

    AG = "AllGather"
    BYP, ADD = mybir.AluOpType.bypass, mybir.AluOpType.add
    MUL, SUB = mybir.AluOpType.mult, mybir.AluOpType.subtract
    AF = mybir.ActivationFunctionType

    with tile.TileContext(nc) as tc:
        with tc.tile_pool(name="abig", bufs=NKB) as apool, \
             tc.tile_pool(name="xtp", bufs=2) as xpool, \
             tc.tile_pool(name="const", bufs=1) as cpool, \
             tc.tile_pool(name="gath", bufs=4) as gpool, \
             tc.tile_pool(name="evict", bufs=3) as epool, \
             tc.tile_pool(name="work", bufs=2) as wpool, \
             tc.tile_pool(name="stage", bufs=4) as spool, \
             tc.tile_pool(name="psum", bufs=5, space="PSUM") as pp, \
             tc.tile_pool(name="psumd", bufs=2, space="PSUM") as ppd, \
             tc.tile_pool(name="psums", bufs=1, space="PSUM") as pps, \
             tc.tile_pool(name="dram", bufs=1, space="DRAM") as dram:

            # ---------- constants ----------
            identity = cpool.tile([P, P], F32)
            nc.sync.dma_start(out=identity[:], in_=ident[:])
            w1_t = cpool.tile([P, DKB * H], BF)
            for kd in range(DKB):
                nc.sync.dma_start(out=w1_t[:, kd * H:(kd + 1) * H],
                                  in_=w1[kd * P:(kd + 1) * P, :])
            wall_t = cpool.tile([P, (H // P) * K3], BF)
            for hcv in range(H // P):
                nc.sync.dma_start(out=wall_t[:, hcv * K3:(hcv + 1) * K3],
                                  in_=wall[hcv * P:(hcv + 1) * P, :])
            wd_t = cpool.tile([K, HD], BF)
            nc.sync.dma_start(out=wd_t[:], in_=wd[:])
            wx_t = cpool.tile([K, D], BF)
            nc.sync.dma_start(out=wx_t[:], in_=wx[:])
            # biases broadcast to all partitions via DMA
            b1_t = cpool.tile([P, H], F32)
            nc.sync.dma_start(out=b1_t[:], in_=b1.ap().to_broadcast([P, H]))
            ball_t = cpool.tile([P, K3], F32)
            nc.sync.dma_start(out=ball_t[:], in_=ball.ap().to_broadcast([P, K3]))
            bd_t = cpool.tile([P, HD], F32)
            nc.sync.dma_start(out=bd_t[:], in_=bd.ap().to_broadcast([P, HD]))
            bx_t = cpool.tile([P, D], F32)
            nc.sync.dma_start(out=bx_t[:], in_=bx.ap().to_broadcast([P, D]))
            # inv_a, inv_b computed redundantly on every partition
            ba_t = cpool.tile([P, K], F32)
            nc.sync.dma_start(out=ba_t[:], in_=beta_a.ap().to_broadcast([P, K]))
            bb_t = cpool.tile([P, K], F32)
            nc.sync.dma_start(out=bb_t[:], in_=beta_b.ap().to_broadcast([P, K]))
            inva = cpool.tile([P, K], F32)
            nc.scalar.activation(inva[:], ba_t[:], AF.Exp)
            nc.scalar.activation(inva[:], inva[:], AF.Ln, bias=1.0)
            nc.vector.reciprocal(inva[:], inva[:])
            invb = cpool.tile([P, K], F32)
            nc.scalar.activation(invb[:], bb_t[:], AF.Exp)
            nc.scalar.activation(invb[:], invb[:], AF.Ln, bias=1.0)
            nc.vector.reciprocal(invb[:], invb[:])
            # replicate along the m axis for the batched elementwise chain
            inva8 = cpool.tile([P, MT, K], F32)
            invb8 = cpool.tile([P, MT, K], F32)
            for m in range(MT):
                nc.vector.tensor_copy(out=inva8[:, m, :], in_=inva[:])
                nc.vector.tensor_copy(out=invb8[:, m, :], in_=invb[:])
            epsb = cpool.tile([P, 1], F32)
            nc.vector.memset(epsb[:], EPS)
            ones_p1 = cpool.tile([P, 1], BF)
            nc.vector.memset(ones_p1[:], 1.0)
            ones_11 = cpool.tile([1, 1], F32)
            nc.vector.memset(ones_11[:], 1.0)

            # ------- phase 1: A load + own degrees + xw -------
            a_res = []
            dch = _chunks(R, 512)
            deg_own_ps = [ppd.tile([1, dw], F32, tag="dacc", name="degown")
                          for (_, dw) in dch]
            for k in range(NKB):
                at = apool.tile([P, R], BF, tag="ablk", name="ablk")
                nc.sync.dma_start(out=at[:], in_=a_cols[k * P:(k + 1) * P, :])
                a_res.append(at)
                # own-node degrees: column sums via ones-matmul
                for ci, (co, cw) in enumerate(dch):
                    nc.tensor.matmul(deg_own_ps[ci][:], ones_p1[:],
                                     at[:, co:co + cw],
                                     start=(k == 0), stop=(k == NKB - 1))

            # own-node d = 1/sqrt(deg) = exp(-0.5 ln(deg)); spread -> [P, MT]
            dow_f = cpool.tile([1, R], F32)
            for ci, (co, cw) in enumerate(dch):
                nc.scalar.activation(dow_f[:, co:co + cw], deg_own_ps[ci][:],
                                     AF.Ln)
            nc.scalar.activation(dow_f[:], dow_f[:], AF.Exp, scale=-0.5)
            down = cpool.tile([P, MT], F32)
            for m in range(MT):
                psd = pps.tile([P, 1], F32, tag="spread", name="spread")
                nc.tensor.matmul(psd[:], dow_f[:, m * P:(m + 1) * P], ones_11[:],
                                 start=True, stop=True)
                nc.vector.tensor_copy(out=down[:, m:m + 1], in_=psd[:])

            # xw = d_own * (x_c @ W1 + b1), bf16  (producer-side scaling)
            cc_xw_in = dram.tile([R, H], BF)
            for g0, gw in _chunks(MT, MG):
                xw_ps = [pp.tile([P, H], F32, tag="acc", name="xwacc")
                         for _ in range(gw)]
                for kd in range(DKB):
                    xt = xpool.tile([P, R], BF, tag="xblk", name="xblk")
                    nc.sync.dma_start(out=xt[:], in_=xT[kd * P:(kd + 1) * P, :])
                    for mi in range(gw):
                        m = g0 + mi
                        nc.tensor.matmul(xw_ps[mi][:], xt[:, m * P:(m + 1) * P],
                                         w1_t[:, kd * H:(kd + 1) * H],
                                         start=(kd == 0), stop=(kd == DKB - 1))
                for mi in range(gw):
                    m = g0 + mi
                    xwf = wpool.tile([P, H], F32, tag="xwf", name="xwf")
                    nc.vector.tensor_tensor(out=xwf[:], in0=xw_ps[mi][:],
                                            in1=b1_t[:], op=ADD)
                    xwb = epool.tile([P, H], BF, tag="ev", name="xwb")
                    nc.vector.tensor_scalar(out=xwb[:], in0=xwf[:],
                                            scalar1=down[:, m:m + 1],
                                            scalar2=None, op0=MUL)
                    nc.gpsimd.dma_start(out=cc_xw_in[m * P:(m + 1) * P, :],
                                        in_=xwb[:])

            # AllGather scaled xw (bf16)
            cc_xw_out = dram.tile([N, H], BF)
            nc.gpsimd.collective_compute(AG, BYP, replica_groups=rg,
                                         ins=[cc_xw_in.opt()],
                                         outs=[cc_xw_out.opt()])

            # ------- phase 2: S1 = (A+I)_cols.T @ YW_full -------
            hT = [cpool.tile([P, R], BF, tag="hT", name="hT", bufs=H // P)
                  for _ in range(H // P)]
            cc_m2_in = dram.tile([R, K3], BF)
            for g0, gw in _chunks(MT, MG):
                s1_acc = [pp.tile([P, H], F32, tag="acc", name="s1acc")
                          for _ in range(gw)]
                for k in range(NKB):
                    gblk = gpool.tile([P, H], BF, tag="gblk", name="gblk")
                    nc.sync.dma_start(out=gblk[:],
                                      in_=cc_xw_out[k * P:(k + 1) * P, :])
                    for mi in range(gw):
                        m = g0 + mi
                        nc.tensor.matmul(s1_acc[mi][:],
                                         a_res[k][:, m * P:(m + 1) * P], gblk[:],
                                         start=(k == 0), stop=(k == NKB - 1))
                # h = leaky_relu(d_own * S1); store transposed bf16; M2 local
                for mi in range(gw):
                    m = g0 + mi
                    htmp = wpool.tile([P, H], F32, tag="htmp", name="htmp")
                    nc.scalar.activation(htmp[:], s1_acc[mi][:], AF.Lrelu,
                                         scale=down[:, m:m + 1], alpha=0.01)
                    for hcv in range(H // P):
                        pst = pp.tile([P, P], F32, tag="acc", name="ptr")
                        nc.tensor.transpose(pst[:], htmp[:, hcv * P:(hcv + 1) * P],
                                            identity[:])
                        nc.vector.tensor_copy(out=hT[hcv][:, m * P:(m + 1) * P],
                                              in_=pst[:])
                    ps2 = pp.tile([P, K3], F32, tag="acc", name="m2acc")
                    for hcv in range(H // P):
                        nc.tensor.matmul(ps2[:], hT[hcv][:, m * P:(m + 1) * P],
                                         wall_t[:, hcv * K3:(hcv + 1) * K3],
                                         start=(hcv == 0),
                                         stop=(hcv == H // P - 1))
                    m2f = wpool.tile([P, K3], F32, tag="m2f", name="m2f")
                    nc.vector.tensor_tensor(out=m2f[:], in0=ps2[:], in1=ball_t[:],
                                            op=ADD)
                    m2b = epool.tile([P, K3], BF, tag="ev", name="m2b")
                    nc.vector.tensor_scalar(out=m2b[:], in0=m2f[:],
                                            scalar1=down[:, m:m + 1],
                                            scalar2=None, op0=MUL)
                    nc.gpsimd.dma_start(out=cc_m2_in[m * P:(m + 1) * P, :],
                                        in_=m2b[:])

            cc_m2_out = dram.tile([N, K3], BF)
            nc.gpsimd.collective_compute(AG, BYP, replica_groups=rg,
                                         ins=[cc_m2_in.opt()],
                                         outs=[cc_m2_out.opt()])

            # ------- phase 3: S2 -------
            s2f8 = cpool.tile([P, MT, K3], F32)
            for g0, gw in _chunks(MT, MG):
                s2_acc = [pp.tile([P, K3], F32, tag="acc", name="s2acc")
                          for _ in range(gw)]
                for k in range(NKB):
                    gblk2 = gpool.tile([P, K3], BF, tag="gblk", name="gblk2")
                    nc.sync.dma_start(out=gblk2[:],
                                      in_=cc_m2_out[k * P:(k + 1) * P, :])
                    for mi in range(gw):
                        m = g0 + mi
                        nc.tensor.matmul(s2_acc[mi][:],
                                         a_res[k][:, m * P:(m + 1) * P],
                                         gblk2[:],
                                         start=(k == 0), stop=(k == NKB - 1))
                for mi in range(gw):
                    m = g0 + mi
                    nc.vector.tensor_scalar(out=s2f8[:, m, :], in0=s2_acc[mi][:],
                                            scalar1=down[:, m:m + 1],
                                            scalar2=None, op0=MUL)

            # ------- batched elementwise chain over all m-tiles -------
            pi_v = s2f8[:, :, 0:K]
            rm_v = s2f8[:, :, K:2 * K]
            rls_v = s2f8[:, :, 2 * K:3 * K]
            ub8 = cpool.tile([P, MT, K], F32)
            nc.sync.dma_start(out=ub8[:],
                              in_=u_beta.ap().rearrange("(m p) t -> p m t", p=P))
            un8 = cpool.tile([P, MT, K], F32)
            nc.sync.dma_start(out=un8[:],
                              in_=u_bern.ap().rearrange("(m p) t -> p m t", p=P))
            er8 = cpool.tile([P, MT, K], F32)
            nc.sync.dma_start(out=er8[:],
                              in_=eps_r.ap().rearrange("(m p) t -> p m t", p=P))
            ca = cpool.tile([P, MT, K], F32)
            cb = cpool.tile([P, MT, K], F32)
            cc_t = cpool.tile([P, MT, K], F32)
            # LV = ln((1 - u^(1/b))^(1/a) + EPS)
            nc.scalar.activation(ca[:], ub8[:], AF.Ln)
            nc.vector.tensor_tensor(out=ca[:], in0=ca[:], in1=invb8[:], op=MUL)
            nc.scalar.activation(ca[:], ca[:], AF.Exp)
            nc.vector.tensor_scalar(out=ca[:], in0=ca[:], scalar1=-1.0,
                                    scalar2=1.0, op0=MUL, op1=ADD)
            nc.scalar.activation(ca[:], ca[:], AF.Ln)
            nc.vector.tensor_tensor(out=ca[:], in0=ca[:], in1=inva8[:], op=MUL)
            nc.scalar.activation(ca[:], ca[:], AF.Exp)
            nc.scalar.activation(ca[:], ca[:], AF.Ln, bias=epsb[:])
            # cumsum along K: Hillis-Steele ping-pong (6 steps for K=64)
            src = ca
            dst = cb
            s = 1
            while s < K:
                nc.vector.tensor_tensor(out=dst[:, :, s:K], in0=src[:, :, s:K],
                                        in1=src[:, :, 0:K - s], op=ADD)
                nc.vector.tensor_copy(out=dst[:, :, 0:s], in_=src[:, :, 0:s])
                src, dst = dst, src
                s *= 2
            cs8 = src                       # cumulative log-prior
            ob = dst                        # free scratch
            # prior = ln(p+EPS) - ln(1+EPS-p), p = exp(cs)
            nc.scalar.activation(ob[:], cs8[:], AF.Exp)
            nc.scalar.activation(cs8[:], ob[:], AF.Ln, bias=epsb[:])
            nc.vector.tensor_scalar(out=ob[:], in0=ob[:], scalar1=-1.0,
                                    scalar2=1.0 + EPS, op0=MUL, op1=ADD)
            nc.scalar.activation(ob[:], ob[:], AF.Ln)
            nc.vector.tensor_tensor(out=cs8[:], in0=cs8[:], in1=ob[:], op=SUB)
            # y = pi + prior + ln(u+EPS) - ln(1+EPS-u);  gate = 1/(1+exp(-y))
            nc.vector.tensor_tensor(out=cs8[:], in0=cs8[:], in1=pi_v, op=ADD)
            nc.scalar.activation(ob[:], un8[:], AF.Ln, bias=epsb[:])
            nc.vector.tensor_tensor(out=cs8[:], in0=cs8[:], in1=ob[:], op=ADD)
            nc.vector.tensor_scalar(out=un8[:], in0=un8[:], scalar1=-1.0,
                                    scalar2=1.0 + EPS, op0=MUL, op1=ADD)
            nc.scalar.activation(un8[:], un8[:], AF.Ln)
            nc.vector.tensor_tensor(out=cs8[:], in0=cs8[:], in1=un8[:], op=SUB)
            nc.scalar.activation(cs8[:], cs8[:], AF.Exp, scale=-1.0)
            nc.vector.tensor_scalar(out=cs8[:], in0=cs8[:], scalar1=1.0,
                                    scalar2=None, op0=ADD)
            nc.vector.reciprocal(cs8[:], cs8[:])
            # r = rm + exp(rls) * eps_r;  z = gate * r
            nc.scalar.activation(cc_t[:], rls_v, AF.Exp)
            nc.vector.tensor_tensor(out=cc_t[:], in0=cc_t[:], in1=er8[:], op=MUL)
            nc.vector.tensor_tensor(out=cc_t[:], in0=cc_t[:], in1=rm_v, op=ADD)
            nc.vector.tensor_tensor(out=cc_t[:], in0=cc_t[:], in1=cs8[:], op=MUL)
            z8 = cc_t
            # transpose z -> zT bf16
            zT = cpool.tile([K, R], BF)
            for m in range(MT):
                psz = pp.tile([P, P], F32, tag="acc", name="ztr")
                nc.tensor.transpose(psz[:K, :], z8[:, m, :], identity[:])
                nc.vector.tensor_copy(out=zT[:, m * P:(m + 1) * P],
                                      in_=psz[:K, :])

            # ------- phase 4: decode -------
            # h2 = z @ Wd + bd, transposed, AllGathered (critical path first)
            h2T = cpool.tile([HD, R], BF)
            cc_h2_in = dram.tile([HD, R], BF)
            for m in range(MT):
                psh = pp.tile([P, HD], F32, tag="acc", name="h2acc")
                nc.tensor.matmul(psh[:], zT[:, m * P:(m + 1) * P], wd_t[:],
                                 start=True, stop=True)
                h2f = wpool.tile([P, HD], F32, tag="h2f", name="h2f")
                nc.vector.tensor_tensor(out=h2f[:], in0=psh[:], in1=bd_t[:],
                                        op=ADD)
                pst2 = pp.tile([P, P], F32, tag="acc", name="h2tr")
                nc.tensor.transpose(pst2[:HD, :], h2f[:], identity[:])
                nc.vector.tensor_copy(out=h2T[:, m * P:(m + 1) * P],
                                      in_=pst2[:HD, :])
            nc.gpsimd.dma_start(out=cc_h2_in[:], in_=h2T[:])
            cc_h2_out = dram.tile([NC * HD, R], BF)
            nc.gpsimd.collective_compute(AG, BYP, replica_groups=rg,
                                         ins=[cc_h2_in.opt()],
                                         outs=[cc_h2_out.opt()])

            # xhat = z @ Wx + bx  (fills PE while the h2 AllGather runs)
            for m in range(MT):
                for (co, cw) in _chunks(D, 512):
                    psx = pp.tile([P, 512], F32, tag="acc", name="xhacc")
                    nc.tensor.matmul(psx[:, :cw], zT[:, m * P:(m + 1) * P],
                                     wx_t[:, co:co + cw], start=True, stop=True)
                    st = spool.tile([P, 512], F32, tag="st", name="stx")
                    nc.vector.tensor_tensor(out=st[:, :cw], in0=psx[:, :cw],
                                            in1=bx_t[:, co:co + cw], op=ADD)
                    nc.sync.dma_start(out=xhat_out[m * P:(m + 1) * P, co:co + cw],
                                      in_=st[:, :cw])

            # edge = h2 @ h2_full.T
            for nr in range(NC):
                for (co, cw) in _chunks(R, 512):
                    rblk = gpool.tile([HD, 512], BF, tag="rblk", name="rblk")
                    nc.sync.dma_start(out=rblk[:, :cw],
                                      in_=cc_h2_out[nr * HD:(nr + 1) * HD,
                                                    co:co + cw])
                    for m in range(MT):
                        pse = pp.tile([P, 512], F32, tag="acc", name="edacc")
                        nc.tensor.matmul(pse[:, :cw], h2T[:, m * P:(m + 1) * P],
                                         rblk[:, :cw], start=True, stop=True)
                        st = spool.tile([P, 512], F32, tag="st", name="ste")
                        nc.vector.tensor_copy(out=st[:, :cw], in_=pse[:, :cw])
                        nc.sync.dma_start(
                            out=edge_out[m * P:(m + 1) * P,
                                         nr * R + co:nr * R + co + cw],
                            in_=st[:, :cw])

    nc.compile()
    return nc


def make_in_maps(cfg: Cfg, inputs):
    N, D, H, K, HD, NC, R = cfg.N, cfg.D, cfg.H, cfg.K, cfg.HD, cfg.NC, cfg.R
    bf = ml_dtypes.bfloat16
    f32 = np.float32
    A = np.asarray(inputs['adj_mat'], f32)
    x = np.asarray(inputs['x'], f32)
    Abf = A.astype(bf)
    xTbf = np.ascontiguousarray(np.asarray(x.T)).astype(bf)
    shared = {
        'w1': np.asarray(inputs['W1'], f32).astype(bf),
        'b1': np.asarray(inputs['b1'], f32).reshape(1, H),
        'wall': np.concatenate([np.asarray(inputs['Wpi'], f32),
                                np.asarray(inputs['Wm'], f32),
                                np.asarray(inputs['Wls'], f32)],
                               axis=1).astype(bf),
        'ball': np.concatenate([np.asarray(inputs['bpi'], f32),
                                np.asarray(inputs['bm'], f32),
                                np.asarray(inputs['bls'], f32)]).reshape(1, 3 * K),
        'wd': np.asarray(inputs['Wd'], f32).astype(bf),
        'bd': np.asarray(inputs['bd'], f32).reshape(1, HD),
        'wx': np.asarray(inputs['Wx'], f32).astype(bf),
        'bx': np.asarray(inputs['bx'], f32).reshape(1, D),
        'beta_a': np.asarray(inputs['beta_a'], f32).reshape(1, K),
        'beta_b': np.asarray(inputs['beta_b'], f32).reshape(1, K),
        'tri': np.triu(np.ones((K, K), f32)),
        'ident': np.eye(P, dtype=f32),
    }
    u_beta = np.asarray(inputs['u_beta'], f32)
    u_bern = np.asarray(inputs['u_bern'], f32)
    eps_r = np.asarray(inputs['eps_r'], f32)
    in_maps = []
    diag = np.arange(R)
    for c in range(NC):
        c0 = c * R
        ac = np.ascontiguousarray(Abf[:, c0:c0 + R])
        ac[c0 + diag, diag] += np.asarray(1.0, bf)   # fold in +I (exact in bf16)
        in_maps.append({
            'a_cols': ac,
            'xT': np.ascontiguousarray(xTbf[:, c0:c0 + R]),
            'u_beta': np.ascontiguousarray(u_beta[c0:c0 + R]),
            'u_bern': np.ascontiguousarray(u_bern[c0:c0 + R]),
            'eps_r': np.ascontiguousarray(eps_r[c0:c0 + R]),
            **shared,
        })
    return in_maps


_CACHE = {}


def _get_nc(cfg: Cfg):
    key = (cfg.N, cfg.D, cfg.H, cfg.K, cfg.HD, cfg.NC)
    if key not in _CACHE:
        _CACHE[key] = build_bass(cfg)
    return _CACHE[key]


def run(cfg: Cfg, inputs, trace=False):
    nc = _get_nc(cfg)
    in_maps = make_in_maps(cfg, inputs)
    res = run_bass_kernel_spmd(nc, in_maps, list(range(cfg.NC)), trace=trace)
    xhat = np.concatenate([r['xhat'] for r in res.results], axis=0)
    edge = np.concatenate([r['edge'] for r in res.results], axis=0)
    return (xhat.reshape(-1).astype(np.float32),
            edge.reshape(-1).astype(np.float32)), res


def kernel(**inputs):
    cfg = Cfg()
    out, _ = run(cfg, inputs, trace=False)
    return out


# revision 11
# speedup vs baseline: 1.4104x; 1.1495x over previous
"""DGLFRM forward pass as a distributed Bass kernel on 8 TRN2 NeuronCores.

Sharding: nodes row-sharded across 8 cores (1024 rows each). adj_mat is
symmetric, so each core loads the COLUMN slice (adj+I)[:, rows_c] in bf16,
which is exactly the transposed-lhs layout the TensorEngine needs for
S = (adj+I) @ Y restricted to its rows. A ones-vector matmul against the
resident slice gives own-row degrees locally (column sums == row sums by
symmetry), so normalization is applied PRODUCER-side: each core scales the
activations it contributes by d_own before the AllGather and row-scales its
matmul outputs by d_own afterwards -- no degree collective and no
core-dependent indexing (the program is pure SPMD). The two activation
AllGathers are split per m-tile group so the second half overlaps the first
half's propagation matmuls. All streaming DMAs move multi-block batches to
amortize descriptor-issue cost on the sync engine.
"""
import sys
if '/opt/trn_rl_repo' not in sys.path:
    sys.path.insert(0, '/opt/trn_rl_repo')

import numpy as np
import ml_dtypes

import concourse.bass as bass
import concourse.bacc as bacc
import concourse.tile as tile
from concourse import mybir
from concourse.bass_utils import run_bass_kernel_spmd

BF = mybir.dt.bfloat16
F32 = mybir.dt.float32
EPS = 1e-7
P = 128


class Cfg:
    def __init__(self, N=8192, D=1024, H=256, K=64, HD=32, NC=8):
        self.N, self.D, self.H, self.K, self.HD, self.NC = N, D, H, K, HD, NC
        self.R = N // NC          # rows per core
        self.NKB = N // P         # k-blocks over full node dim
        self.MT = self.R // P     # m-tiles per core
        self.DKB = D // P         # k-blocks over feature dim


def _chunks(total, step):
    out = []
    o = 0
    while o < total:
        out.append((o, min(step, total - o)))
        o += step
    return out


def build_bass(cfg: Cfg):
    N, D, H, K, HD, NC = cfg.N, cfg.D, cfg.H, cfg.K, cfg.HD, cfg.NC
    R, NKB, MT, DKB = cfg.R, cfg.NKB, cfg.MT, cfg.DKB
    K3 = 3 * K
    MG = min(MT, 4)               # m-tiles per accumulation group
    AB = min(8, NKB)              # A-blocks per load batch
    groups = _chunks(MT, MG)
    rg = [list(range(NC))]

    nc = bacc.Bacc("TRN2", target_bir_lowering=False, debug=False, num_devices=NC)

    # ---- I/O ----
    a_cols = nc.declare_dram_parameter("a_cols", [N, R], BF, isOutput=False)
    xT = nc.declare_dram_parameter("xT", [D, R], BF, isOutput=False)
    w1 = nc.declare_dram_parameter("w1", [D, H], BF, isOutput=False)
    b1 = nc.declare_dram_parameter("b1", [1, H], F32, isOutput=False)
    wall = nc.declare_dram_parameter("wall", [H, K3], BF, isOutput=False)
    ball = nc.declare_dram_parameter("ball", [1, K3], F32, isOutput=False)
    wd = nc.declare_dram_parameter("wd", [K, HD], BF, isOutput=False)
    bd = nc.declare_dram_parameter("bd", [1, HD], F32, isOutput=False)
    wx = nc.declare_dram_parameter("wx", [K, D], BF, isOutput=False)
    bx = nc.declare_dram_parameter("bx", [1, D], F32, isOutput=False)
    beta_a = nc.declare_dram_parameter("beta_a", [1, K], F32, isOutput=False)
    beta_b = nc.declare_dram_parameter("beta_b", [1, K], F32, isOutput=False)
    u_beta = nc.declare_dram_parameter("u_beta", [R, K], F32, isOutput=False)
    u_bern = nc.declare_dram_parameter("u_bern", [R, K], F32, isOutput=False)
    eps_r = nc.declare_dram_parameter("eps_r", [R, K], F32, isOutput=False)
    tri = nc.declare_dram_parameter("tri", [K, K], F32, isOutput=False)
    ident = nc.declare_dram_parameter("ident", [P, P], F32, isOutput=False)
    xhat_out = nc.declare_dram_parameter("xhat", [R, D], F32, isOutput=True)
    edge_out = nc.declare_dram_parameter("edge", [R, N], F32, isOutput=True)

    AG = "AllGather"
    BYP, ADD = mybir.AluOpType.bypass, mybir.AluOpType.add
    MUL, SUB = mybir.AluOpType.mult, mybir.AluOpType.subtract
    AF = mybir.ActivationFunctionType

    def ablk(k):
        # lhsT slice accessor for global k-block
        return a_res[k // AB][:, k % AB, :]

    with tile.TileContext(nc) as tc:
        with tc.tile_pool(name="abig", bufs=max(1, NKB // AB)) as apool, \
             tc.tile_pool(name="xtp", bufs=2) as xpool, \
             tc.tile_pool(name="const", bufs=1) as cpool, \
             tc.tile_pool(name="gath", bufs=3) as gpool, \
             tc.tile_pool(name="evict", bufs=3) as epool, \
             tc.tile_pool(name="work", bufs=2) as wpool, \
             tc.tile_pool(name="stage", bufs=2) as spool, \
             tc.tile_pool(name="psum", bufs=5, space="PSUM") as pp, \
             tc.tile_pool(name="psumd", bufs=2, space="PSUM") as ppd, \
             tc.tile_pool(name="psums", bufs=1, space="PSUM") as pps, \
             tc.tile_pool(name="dram", bufs=1, space="DRAM") as dram:

            # ---------- constants ----------
            identity = cpool.tile([P, P], F32)
            nc.sync.dma_start(out=identity[:], in_=ident[:])
            w1_t = cpool.tile([P, DKB * H], BF)
            for kd in range(DKB):
                nc.sync.dma_start(out=w1_t[:, kd * H:(kd + 1) * H],
                                  in_=w1[kd * P:(kd + 1) * P, :])
            wall_t = cpool.tile([P, (H // P) * K3], BF)
            for hcv in range(H // P):
                nc.sync.dma_start(out=wall_t[:, hcv * K3:(hcv + 1) * K3],
                                  in_=wall[hcv * P:(hcv + 1) * P, :])
            wd_t = cpool.tile([K, HD], BF)
            nc.sync.dma_start(out=wd_t[:], in_=wd[:])
            wx_t = cpool.tile([K, D], BF)
            nc.sync.dma_start(out=wx_t[:], in_=wx[:])
            b1_t = cpool.tile([P, H], F32)
            nc.sync.dma_start(out=b1_t[:], in_=b1.ap().to_broadcast([P, H]))
            ball_t = cpool.tile([P, K3], F32)
            nc.sync.dma_start(out=ball_t[:], in_=ball.ap().to_broadcast([P, K3]))
            bd_t = cpool.tile([P, HD], F32)
            nc.sync.dma_start(out=bd_t[:], in_=bd.ap().to_broadcast([P, HD]))
            bx_t = cpool.tile([P, D], F32)
            nc.sync.dma_start(out=bx_t[:], in_=bx.ap().to_broadcast([P, D]))
            ba_t = cpool.tile([P, K], F32)
            nc.sync.dma_start(out=ba_t[:], in_=beta_a.ap().to_broadcast([P, K]))
            bb_t = cpool.tile([P, K], F32)
            nc.sync.dma_start(out=bb_t[:], in_=beta_b.ap().to_broadcast([P, K]))
            inva = cpool.tile([P, K], F32)
            nc.scalar.activation(inva[:], ba_t[:], AF.Exp)
            nc.scalar.activation(inva[:], inva[:], AF.Ln, bias=1.0)
            nc.vector.reciprocal(inva[:], inva[:])
            invb = cpool.tile([P, K], F32)
            nc.scalar.activation(invb[:], bb_t[:], AF.Exp)
            nc.scalar.activation(invb[:], invb[:], AF.Ln, bias=1.0)
            nc.vector.reciprocal(invb[:], invb[:])
            inva8 = cpool.tile([P, MT, K], F32)
            invb8 = cpool.tile([P, MT, K], F32)
            for m in range(MT):
                nc.vector.tensor_copy(out=inva8[:, m, :], in_=inva[:])
                nc.vector.tensor_copy(out=invb8[:, m, :], in_=invb[:])
            epsb = cpool.tile([P, 1], F32)
            nc.vector.memset(epsb[:], EPS)
            ones_p1 = cpool.tile([P, 1], BF)
            nc.vector.memset(ones_p1[:], 1.0)
            ones_11 = cpool.tile([1, 1], F32)
            nc.vector.memset(ones_11[:], 1.0)

            # ------- phase 1: batched A load + own degrees + xw -------
            a_res = []
            dch = _chunks(R, 512)
            deg_own_ps = [ppd.tile([1, dw], F32, tag="dacc", name="degown")
                          for (_, dw) in dch]
            for g in range(NKB // AB):
                at = apool.tile([P, AB, R], BF, tag="ablk", name="ablk")
                nc.sync.dma_start(
                    out=at[:],
                    in_=a_cols[g * AB * P:(g + 1) * AB * P, :]
                    .rearrange("(k p) c -> p k c", p=P))
                a_res.append(at)
                for j in range(AB):
                    k = g * AB + j
                    for ci, (co, cw) in enumerate(dch):
                        nc.tensor.matmul(deg_own_ps[ci][:], ones_p1[:],
                                         at[:, j, co:co + cw],
                                         start=(k == 0), stop=(k == NKB - 1))

            # d_own = exp(-0.5 ln(deg)); spread [1,R] -> [P, MT]
            dow_f = cpool.tile([1, R], F32)
            for ci, (co, cw) in enumerate(dch):
                nc.scalar.activation(dow_f[:, co:co + cw], deg_own_ps[ci][:],
                                     AF.Ln)
            nc.scalar.activation(dow_f[:], dow_f[:], AF.Exp, scale=-0.5)
            down = cpool.tile([P, MT], F32)
            for m in range(MT):
                psd = pps.tile([P, 1], F32, tag="spread", name="spread")
                nc.tensor.matmul(psd[:], dow_f[:, m * P:(m + 1) * P], ones_11[:],
                                 start=True, stop=True)
                nc.vector.tensor_copy(out=down[:, m:m + 1], in_=psd[:])

            # xw = d_own * (x_c @ W1 + b1), bf16; AllGather split per group
            cc_xw_in = [dram.tile([gw * P, H], BF, tag="ccxi", name="ccxi",
                                  bufs=len(groups)) for (_, gw) in groups]
            cc_xw_out = [dram.tile([NC * gw * P, H], BF, tag="ccxo", name="ccxo",
                                   bufs=len(groups)) for (_, gw) in groups]
            for gi, (g0, gw) in enumerate(groups):
                xw_ps = [pp.tile([P, H], F32, tag="acc", name="xwacc")
                         for _ in range(gw)]
                for kd in range(DKB):
                    xt = xpool.tile([P, R], BF, tag="xblk", name="xblk")
                    nc.sync.dma_start(out=xt[:], in_=xT[kd * P:(kd + 1) * P, :])
                    for mi in range(gw):
                        m = g0 + mi
                        nc.tensor.matmul(xw_ps[mi][:], xt[:, m * P:(m + 1) * P],
                                         w1_t[:, kd * H:(kd + 1) * H],
                                         start=(kd == 0), stop=(kd == DKB - 1))
                for mi in range(gw):
                    m = g0 + mi
                    xwf = wpool.tile([P, H], F32, tag="xwf", name="xwf")
                    nc.vector.tensor_tensor(out=xwf[:], in0=xw_ps[mi][:],
                                            in1=b1_t[:], op=ADD)
                    xwb = epool.tile([P, H], BF, tag="ev", name="xwb")
                    nc.vector.tensor_scalar(out=xwb[:], in0=xwf[:],
                                            scalar1=down[:, m:m + 1],
                                            scalar2=None, op0=MUL)
                    nc.gpsimd.dma_start(out=cc_xw_in[gi][mi * P:(mi + 1) * P, :],
                                        in_=xwb[:])
                nc.gpsimd.collective_compute(AG, BYP, replica_groups=rg,
                                             ins=[cc_xw_in[gi].opt()],
                                             outs=[cc_xw_out[gi].opt()])

            # ------- phase 2: S1 = (A+I)_cols.T @ YW_full -------
            # gathered k-block for (AG-chunk gi, rank r, sub jb):
            #   global k = r*MT + g0[gi] + jb ; rows (r*gw+jb)*P of cc_xw_out[gi]
            hT = [cpool.tile([P, R], BF, tag="hT", name="hT", bufs=H // P)
                  for _ in range(H // P)]
            cc_m2_in = [dram.tile([gw * P, K3], BF, tag="ccmi", name="ccmi",
                                  bufs=len(groups)) for (_, gw) in groups]
            cc_m2_out = [dram.tile([NC * gw * P, K3], BF, tag="ccmo", name="ccmo",
                                   bufs=len(groups)) for (_, gw) in groups]

            def prop_pass(cc_out_list, width, acc_w, evict_fn):
                """Two grouped accumulation passes over all gathered blocks."""
                for g0a, gwa in groups:          # accumulation m-groups
                    accs = [pp.tile([P, acc_w], F32, tag="acc", name="pacc")
                            for _ in range(gwa)]
                    ki = 0
                    for gi, (g0, gw) in enumerate(groups):   # AG chunks
                        for r in range(NC):
                            gb = gpool.tile([P, gw, width], BF, tag="gblk",
                                            name="gblk")
                            nc.sync.dma_start(
                                out=gb[:],
                                in_=cc_out_list[gi][r * gw * P:(r + 1) * gw * P, :]
                                .rearrange("(k p) c -> p k c", p=P))
                            for jb in range(gw):
                                k = r * MT + g0 + jb
                                for mi in range(gwa):
                                    m = g0a + mi
                                    nc.tensor.matmul(
                                        accs[mi][:], ablk(k)[:, m * P:(m + 1) * P],
                                        gb[:, jb, :], start=(ki == 0),
                                        stop=(ki == NKB - 1))
                                ki += 1
                    for mi in range(gwa):
                        evict_fn(g0a + mi, accs[mi])

            def s1_evict(m, acc):
                gi, off = m // MG, m % MG
                htmp = wpool.tile([P, H], F32, tag="htmp", name="htmp")
                nc.scalar.activation(htmp[:], acc[:], AF.Lrelu,
                                     scale=down[:, m:m + 1], alpha=0.01)
                for hcv in range(H // P):
                    pst = pp.tile([P, P], F32, tag="acc", name="ptr")
                    nc.tensor.transpose(pst[:], htmp[:, hcv * P:(hcv + 1) * P],
                                        identity[:])
                    nc.vector.tensor_copy(out=hT[hcv][:, m * P:(m + 1) * P],
                                          in_=pst[:])
                ps2 = pp.tile([P, K3], F32, tag="acc", name="m2acc")
                for hcv in range(H // P):
                    nc.tensor.matmul(ps2[:], hT[hcv][:, m * P:(m + 1) * P],
                                     wall_t[:, hcv * K3:(hcv + 1) * K3],
                                     start=(hcv == 0), stop=(hcv == H // P - 1))
                m2f = wpool.tile([P, K3], F32, tag="m2f", name="m2f")
                nc.vector.tensor_tensor(out=m2f[:], in0=ps2[:], in1=ball_t[:],
                                        op=ADD)
                m2b = epool.tile([P, K3], BF, tag="ev", name="m2b")
                nc.vector.tensor_scalar(out=m2b[:], in0=m2f[:],
                                        scalar1=down[:, m:m + 1],
                                        scalar2=None, op0=MUL)
                nc.gpsimd.dma_start(out=cc_m2_in[gi][off * P:(off + 1) * P, :],
                                    in_=m2b[:])
                if off == (groups[gi][1] - 1):   # last m of this AG chunk
                    nc.gpsimd.collective_compute(
                        AG, BYP, replica_groups=rg,
                        ins=[cc_m2_in[gi].opt()], outs=[cc_m2_out[gi].opt()])

            prop_pass(cc_xw_out, H, H, s1_evict)

            # ------- phase 3: S2 -------
            s2f8 = cpool.tile([P, MT, K3], F32)

            def s2_evict(m, acc):
                nc.vector.tensor_scalar(out=s2f8[:, m, :], in0=acc[:],
                                        scalar1=down[:, m:m + 1],
                                        scalar2=None, op0=MUL)

            prop_pass(cc_m2_out, K3, K3, s2_evict)

            # ------- batched elementwise chain over all m-tiles -------
            pi_v = s2f8[:, :, 0:K]
            rm_v = s2f8[:, :, K:2 * K]
            rls_v = s2f8[:, :, 2 * K:3 * K]
            ca = cpool.tile([P, MT, K], F32)
            nc.sync.dma_start(out=ca[:],
                              in_=u_beta.ap().rearrange("(m p) t -> p m t", p=P))
            un8 = cpool.tile([P, MT, K], F32)
            nc.sync.dma_start(out=un8[:],
                              in_=u_bern.ap().rearrange("(m p) t -> p m t", p=P))
            er8 = cpool.tile([P, MT, K], F32)
            nc.sync.dma_start(out=er8[:],
                              in_=eps_r.ap().rearrange("(m p) t -> p m t", p=P))
            cb = cpool.tile([P, MT, K], F32)
            cc_t = cpool.tile([P, MT, K], F32)
            # LV = ln((1 - u^(1/b))^(1/a) + EPS)   (in-place in ca)
            nc.scalar.activation(ca[:], ca[:], AF.Ln)
            nc.vector.tensor_tensor(out=ca[:], in0=ca[:], in1=invb8[:], op=MUL)
            nc.scalar.activation(ca[:], ca[:], AF.Exp)
            nc.vector.tensor_scalar(out=ca[:], in0=ca[:], scalar1=-1.0,
                                    scalar2=1.0, op0=MUL, op1=ADD)
            nc.scalar.activation(ca[:], ca[:], AF.Ln)
            nc.vector.tensor_tensor(out=ca[:], in0=ca[:], in1=inva8[:], op=MUL)
            nc.scalar.activation(ca[:], ca[:], AF.Exp)
            nc.scalar.activation(ca[:], ca[:], AF.Ln, bias=epsb[:])
            # cumsum along K: Hillis-Steele ping-pong
            src, dst = ca, cb
            s = 1
            while s < K:
                nc.vector.tensor_tensor(out=dst[:, :, s:K], in0=src[:, :, s:K],
                                        in1=src[:, :, 0:K - s], op=ADD)
                nc.vector.tensor_copy(out=dst[:, :, 0:s], in_=src[:, :, 0:s])
                src, dst = dst, src
                s *= 2
            cs8, ob = src, dst
            # prior = ln(p+EPS) - ln(1+EPS-p), p = exp(cs)
            nc.scalar.activation(ob[:], cs8[:], AF.Exp)
            nc.scalar.activation(cs8[:], ob[:], AF.Ln, bias=epsb[:])
            nc.vector.tensor_scalar(out=ob[:], in0=ob[:], scalar1=-1.0,
                                    scalar2=1.0 + EPS, op0=MUL, op1=ADD)
            nc.scalar.activation(ob[:], ob[:], AF.Ln)
            nc.vector.tensor_tensor(out=cs8[:], in0=cs8[:], in1=ob[:], op=SUB)
            # y = pi + prior + ln(u+EPS) - ln(1+EPS-u); gate = 1/(1+exp(-y))
            nc.vector.tensor_tensor(out=cs8[:], in0=cs8[:], in1=pi_v, op=ADD)
            nc.scalar.activation(ob[:], un8[:], AF.Ln, bias=epsb[:])
            nc.vector.tensor_tensor(out=cs8[:], in0=cs8[:], in1=ob[:], op=ADD)
            nc.vector.tensor_scalar(out=un8[:], in0=un8[:], scalar1=-1.0,
                                    scalar2=1.0 + EPS, op0=MUL, op1=ADD)
            nc.scalar.activation(un8[:], un8[:], AF.Ln)
            nc.vector.tensor_tensor(out=cs8[:], in0=cs8[:], in1=un8[:], op=SUB)
            nc.scalar.activation(cs8[:], cs8[:], AF.Exp, scale=-1.0)
            nc.vector.tensor_scalar(out=cs8[:], in0=cs8[:], scalar1=1.0,
                                    scalar2=None, op0=ADD)
            nc.vector.reciprocal(cs8[:], cs8[:])
            # r = rm + exp(rls)*eps_r;  z = gate * r
            nc.scalar.activation(cc_t[:], rls_v, AF.Exp)
            nc.vector.tensor_tensor(out=cc_t[:], in0=cc_t[:], in1=er8[:], op=MUL)
            nc.vector.tensor_tensor(out=cc_t[:], in0=cc_t[:], in1=rm_v, op=ADD)
            nc.vector.tensor_tensor(out=cc_t[:], in0=cc_t[:], in1=cs8[:], op=MUL)
            z8 = cc_t
            zT = cpool.tile([K, R], BF)
            for m in range(MT):
                psz = pp.tile([P, P], F32, tag="acc", name="ztr")
                nc.tensor.transpose(psz[:K, :], z8[:, m, :], identity[:])
                nc.vector.tensor_copy(out=zT[:, m * P:(m + 1) * P],
                                      in_=psz[:K, :])

            # ------- phase 4: decode -------
            h2T = cpool.tile([HD, R], BF)
            cc_h2_in = dram.tile([HD, R], BF)
            for m in range(MT):
                psh = pp.tile([P, HD], F32, tag="acc", name="h2acc")
                nc.tensor.matmul(psh[:], zT[:, m * P:(m + 1) * P], wd_t[:],
                                 start=True, stop=True)
                h2f = wpool.tile([P, HD], F32, tag="h2f", name="h2f")
                nc.vector.tensor_tensor(out=h2f[:], in0=psh[:], in1=bd_t[:],
                                        op=ADD)
                pst2 = pp.tile([P, P], F32, tag="acc", name="h2tr")
                nc.tensor.transpose(pst2[:HD, :], h2f[:], identity[:])
                nc.vector.tensor_copy(out=h2T[:, m * P:(m + 1) * P],
                                      in_=pst2[:HD, :])
            nc.gpsimd.dma_start(out=cc_h2_in[:], in_=h2T[:])
            cc_h2_out = dram.tile([NC * HD, R], BF)
            nc.gpsimd.collective_compute(AG, BYP, replica_groups=rg,
                                         ins=[cc_h2_in.opt()],
                                         outs=[cc_h2_out.opt()])

            # xhat = z @ Wx + bx  (fills PE while the h2 AllGather runs)
            for m in range(MT):
                for (bo, bw) in _chunks(D, 1024):
                    stx = spool.tile([P, 1024], F32, tag="st", name="stx")
                    for (co, cw) in _chunks(bw, 512):
                        psx = pp.tile([P, 512], F32, tag="acc", name="xhacc")
                        nc.tensor.matmul(psx[:, :cw], zT[:, m * P:(m + 1) * P],
                                         wx_t[:, bo + co:bo + co + cw],
                                         start=True, stop=True)
                        nc.vector.tensor_tensor(out=stx[:, co:co + cw],
                                                in0=psx[:, :cw],
                                                in1=bx_t[:, bo + co:bo + co + cw],
                                                op=ADD)
                    nc.sync.dma_start(out=xhat_out[m * P:(m + 1) * P, bo:bo + bw],
                                      in_=stx[:, :bw])

            # edge = h2 @ h2_full.T  (batched 512KB stores)
            for nr in range(NC):
                rblk = gpool.tile([HD, R], BF, tag="rblk", name="rblk")
                nc.sync.dma_start(out=rblk[:],
                                  in_=cc_h2_out[nr * HD:(nr + 1) * HD, :])
                for m in range(MT):
                    ste = spool.tile([P, 1024], F32, tag="st", name="ste")
                    for (co, cw) in _chunks(R, 512):
                        pse = pp.tile([P, 512], F32, tag="acc", name="edacc")
                        nc.tensor.matmul(pse[:, :cw], h2T[:, m * P:(m + 1) * P],
                                         rblk[:, co:co + cw],
                                         start=True, stop=True)
                        nc.vector.tensor_copy(out=ste[:, co:co + cw],
                                              in_=pse[:, :cw])
                    nc.sync.dma_start(
                        out=edge_out[m * P:(m + 1) * P, nr * R:nr * R + R],
                        in_=ste[:, :R])

    nc.compile()
    return nc


def make_in_maps(cfg: Cfg, inputs):
    N, D, H, K, HD, NC, R = cfg.N, cfg.D, cfg.H, cfg.K, cfg.HD, cfg.NC, cfg.R
    bf = ml_dtypes.bfloat16
    f32 = np.float32
    A = np.asarray(inputs['adj_mat'], f32)
    x = np.asarray(inputs['x'], f32)
    Abf = A.astype(bf)
    xTbf = np.ascontiguousarray(np.asarray(x.T)).astype(bf)
    shared = {
        'w1': np.asarray(inputs['W1'], f32).astype(bf),
        'b1': np.asarray(inputs['b1'], f32).reshape(1, H),
        'wall': np.concatenate([np.asarray(inputs['Wpi'], f32),
                                np.asarray(inputs['Wm'], f32),
                                np.asarray(inputs['Wls'], f32)],
                               axis=1).astype(bf),
        'ball': np.concatenate([np.asarray(inputs['bpi'], f32),
                                np.asarray(inputs['bm'], f32),
                                np.asarray(inputs['bls'], f32)]).reshape(1, 3 * K),
        'wd': np.asarray(inputs['Wd'], f32).astype(bf),
        'bd': np.asarray(inputs['bd'], f32).reshape(1, HD),
        'wx': np.asarray(inputs['Wx'], f32).astype(bf),
        'bx': np.asarray(inputs['bx'], f32).reshape(1, D),
        'beta_a': np.asarray(inputs['beta_a'], f32).reshape(1, K),
        'beta_b': np.asarray(inputs['beta_b'], f32).reshape(1, K),
        'tri': np.triu(np.ones((K, K), f32)),
        'ident': np.eye(P, dtype=f32),
    }
    u_beta = np.asarray(inputs['u_beta'], f32)
    u_bern = np.asarray(inputs['u_bern'], f32)
    eps_r = np.asarray(inputs['eps_r'], f32)
    in_maps = []
    diag = np.arange(R)
    for c in range(NC):
        c0 = c * R
        ac = np.ascontiguousarray(Abf[:, c0:c0 + R])
        ac[c0 + diag, diag] += np.asarray(1.0, bf)   # fold in +I (exact in bf16)
        in_maps.append({
            'a_cols': ac,
            'xT': np.ascontiguousarray(xTbf[:, c0:c0 + R]),
            'u_beta': np.ascontiguousarray(u_beta[c0:c0 + R]),
            'u_bern': np.ascontiguousarray(u_bern[c0:c0 + R]),
            'eps_r': np.ascontiguousarray(eps_r[c0:c0 + R]),
            **shared,
        })
    return in_maps


_CACHE = {}


def _get_nc(cfg: Cfg):
    key = (cfg.N, cfg.D, cfg.H, cfg.K, cfg.HD, cfg.NC)
    if key not in _CACHE:
        _CACHE[key] = build_bass(cfg)
    return _CACHE[key]


def run(cfg: Cfg, inputs, trace=False):
    nc = _get_nc(cfg)
    in_maps = make_in_maps(cfg, inputs)
    res = run_bass_kernel_spmd(nc, in_maps, list(range(cfg.NC)), trace=trace)
    xhat = np.concatenate([r['xhat'] for r in res.results], axis=0)
    edge = np.concatenate([r['edge'] for r in res.results], axis=0)
    return (xhat.reshape(-1).astype(np.float32),
            edge.reshape(-1).astype(np.float32)), res


def kernel(**inputs):
    cfg = Cfg()
    out, _ = run(cfg, inputs, trace=False)
    return out


# revision 12
# speedup vs baseline: 1.4675x; 1.0405x over previous
"""DGLFRM forward pass as a distributed Bass kernel on 8 TRN2 NeuronCores.

Sharding: nodes row-sharded across 8 cores (1024 rows each). adj_mat is
symmetric, so each core loads the COLUMN slice (adj+I)[:, rows_c] in bf16,
which is exactly the transposed-lhs layout the TensorEngine needs for
S = (adj+I) @ Y restricted to its rows. A ones-vector matmul against the
resident slice gives own-row degrees locally (column sums == row sums by
symmetry), so normalization is applied PRODUCER-side: each core scales the
activations it contributes by d_own before the AllGather and row-scales its
matmul outputs by d_own afterwards -- no degree collective and no
core-dependent indexing (the program is pure SPMD). The two activation
AllGathers are split per m-tile group so the second half overlaps the first
half's propagation matmuls. All streaming DMAs move multi-block batches to
amortize descriptor-issue cost on the sync engine.
"""
import sys
if '/opt/trn_rl_repo' not in sys.path:
    sys.path.insert(0, '/opt/trn_rl_repo')

import numpy as np
import ml_dtypes

import concourse.bass as bass
import concourse.bacc as bacc
import concourse.tile as tile
from concourse import mybir
from concourse.bass_utils import run_bass_kernel_spmd

BF = mybir.dt.bfloat16
F32 = mybir.dt.float32
EPS = 1e-7
P = 128


class Cfg:
    def __init__(self, N=8192, D=1024, H=256, K=64, HD=32, NC=8):
        self.N, self.D, self.H, self.K, self.HD, self.NC = N, D, H, K, HD, NC
        self.R = N // NC          # rows per core
        self.NKB = N // P         # k-blocks over full node dim
        self.MT = self.R // P     # m-tiles per core
        self.DKB = D // P         # k-blocks over feature dim


def _chunks(total, step):
    out = []
    o = 0
    while o < total:
        out.append((o, min(step, total - o)))
        o += step
    return out


def build_bass(cfg: Cfg):
    N, D, H, K, HD, NC = cfg.N, cfg.D, cfg.H, cfg.K, cfg.HD, cfg.NC
    R, NKB, MT, DKB = cfg.R, cfg.NKB, cfg.MT, cfg.DKB
    K3 = 3 * K
    MG = min(MT, 4)               # m-tiles per accumulation group
    AB = min(8, NKB)              # A-blocks per load batch
    groups = _chunks(MT, MG)
    rg = [list(range(NC))]

    nc = bacc.Bacc("TRN2", target_bir_lowering=False, debug=False, num_devices=NC)

    # ---- I/O ----
    a_cols = nc.declare_dram_parameter("a_cols", [P, NKB * R], BF,
                                      isOutput=False)
    xT = nc.declare_dram_parameter("xT", [D, R], BF, isOutput=False)
    w1 = nc.declare_dram_parameter("w1", [D, H], BF, isOutput=False)
    b1 = nc.declare_dram_parameter("b1", [1, H], F32, isOutput=False)
    wall = nc.declare_dram_parameter("wall", [H, K3], BF, isOutput=False)
    ball = nc.declare_dram_parameter("ball", [1, K3], F32, isOutput=False)
    wd = nc.declare_dram_parameter("wd", [K, HD], BF, isOutput=False)
    bd = nc.declare_dram_parameter("bd", [1, HD], F32, isOutput=False)
    wx = nc.declare_dram_parameter("wx", [K, D], BF, isOutput=False)
    bx = nc.declare_dram_parameter("bx", [1, D], F32, isOutput=False)
    beta_a = nc.declare_dram_parameter("beta_a", [1, K], F32, isOutput=False)
    beta_b = nc.declare_dram_parameter("beta_b", [1, K], F32, isOutput=False)
    u_beta = nc.declare_dram_parameter("u_beta", [R, K], F32, isOutput=False)
    u_bern = nc.declare_dram_parameter("u_bern", [R, K], F32, isOutput=False)
    eps_r = nc.declare_dram_parameter("eps_r", [R, K], F32, isOutput=False)
    tri = nc.declare_dram_parameter("tri", [K, K], F32, isOutput=False)
    ident = nc.declare_dram_parameter("ident", [P, P], F32, isOutput=False)
    xhat_out = nc.declare_dram_parameter("xhat", [R, D], F32, isOutput=True)
    edge_out = nc.declare_dram_parameter("edge", [R, N], F32, isOutput=True)

    AG = "AllGather"
    BYP, ADD = mybir.AluOpType.bypass, mybir.AluOpType.add
    MUL, SUB = mybir.AluOpType.mult, mybir.AluOpType.subtract
    AF = mybir.ActivationFunctionType

    def ablk(k):
        # lhsT slice accessor for global k-block
        return a_res[k // AB][:, (k % AB) * R:(k % AB + 1) * R]

    with tile.TileContext(nc) as tc:
        with tc.tile_pool(name="abig", bufs=max(1, NKB // AB)) as apool, \
             tc.tile_pool(name="xtp", bufs=2) as xpool, \
             tc.tile_pool(name="const", bufs=1) as cpool, \
             tc.tile_pool(name="gath", bufs=3) as gpool, \
             tc.tile_pool(name="evict", bufs=3) as epool, \
             tc.tile_pool(name="work", bufs=2) as wpool, \
             tc.tile_pool(name="stage", bufs=2) as spool, \
             tc.tile_pool(name="psum", bufs=6, space="PSUM") as pp, \
             tc.tile_pool(name="psumd", bufs=2, space="PSUM") as ppd, \
             tc.tile_pool(name="dram", bufs=1, space="DRAM") as dram:

            # ---------- constants ----------
            identity = cpool.tile([P, P], F32)
            nc.sync.dma_start(out=identity[:], in_=ident[:])
            w1_t = cpool.tile([P, DKB * H], BF)
            for kd in range(DKB):
                nc.sync.dma_start(out=w1_t[:, kd * H:(kd + 1) * H],
                                  in_=w1[kd * P:(kd + 1) * P, :])
            wall_t = cpool.tile([P, (H // P) * K3], BF)
            for hcv in range(H // P):
                nc.sync.dma_start(out=wall_t[:, hcv * K3:(hcv + 1) * K3],
                                  in_=wall[hcv * P:(hcv + 1) * P, :])
            wd_t = cpool.tile([K, HD], BF)
            nc.sync.dma_start(out=wd_t[:], in_=wd[:])
            wx_t = cpool.tile([K, D], BF)
            nc.sync.dma_start(out=wx_t[:], in_=wx[:])
            b1_t = cpool.tile([P, H], F32)
            nc.sync.dma_start(out=b1_t[:], in_=b1.ap().to_broadcast([P, H]))
            ball_t = cpool.tile([P, K3], F32)
            nc.sync.dma_start(out=ball_t[:], in_=ball.ap().to_broadcast([P, K3]))
            bd_t = cpool.tile([P, HD], F32)
            nc.sync.dma_start(out=bd_t[:], in_=bd.ap().to_broadcast([P, HD]))
            bx_t = cpool.tile([P, D], F32)
            nc.sync.dma_start(out=bx_t[:], in_=bx.ap().to_broadcast([P, D]))
            ba_t = cpool.tile([P, K], F32)
            nc.sync.dma_start(out=ba_t[:], in_=beta_a.ap().to_broadcast([P, K]))
            bb_t = cpool.tile([P, K], F32)
            nc.sync.dma_start(out=bb_t[:], in_=beta_b.ap().to_broadcast([P, K]))
            inva = cpool.tile([P, K], F32)
            nc.scalar.activation(inva[:], ba_t[:], AF.Exp)
            nc.scalar.activation(inva[:], inva[:], AF.Ln, bias=1.0)
            nc.vector.reciprocal(inva[:], inva[:])
            invb = cpool.tile([P, K], F32)
            nc.scalar.activation(invb[:], bb_t[:], AF.Exp)
            nc.scalar.activation(invb[:], invb[:], AF.Ln, bias=1.0)
            nc.vector.reciprocal(invb[:], invb[:])
            inva8 = cpool.tile([P, MT, K], F32)
            invb8 = cpool.tile([P, MT, K], F32)
            for m in range(MT):
                nc.vector.tensor_copy(out=inva8[:, m, :], in_=inva[:])
                nc.vector.tensor_copy(out=invb8[:, m, :], in_=invb[:])
            epsb = cpool.tile([P, 1], F32)
            nc.vector.memset(epsb[:], EPS)
            ones_p1 = cpool.tile([P, 1], BF)
            nc.vector.memset(ones_p1[:], 1.0)
            ones_11 = cpool.tile([1, 1], F32)
            nc.vector.memset(ones_11[:], 1.0)

            # ------- phase 1: batched A load + own degrees + xw -------
            a_res = []
            dch = _chunks(R, 512)
            deg_own_ps = [ppd.tile([1, dw], F32, tag="dacc", name="degown")
                          for (_, dw) in dch]
            for g in range(NKB // AB):
                at = apool.tile([P, AB * R], BF, tag="ablk", name="ablk")
                nc.sync.dma_start(out=at[:],
                                  in_=a_cols[:, g * AB * R:(g + 1) * AB * R])
                a_res.append(at)
                for j in range(AB):
                    k = g * AB + j
                    for ci, (co, cw) in enumerate(dch):
                        nc.tensor.matmul(deg_own_ps[ci][:], ones_p1[:],
                                         at[:, j * R + co:j * R + co + cw],
                                         start=(k == 0), stop=(k == NKB - 1))

            # d_own = exp(-0.5 ln(deg)); spread [1,R] -> [P, MT]
            dow_f = cpool.tile([1, R], F32)
            for ci, (co, cw) in enumerate(dch):
                nc.scalar.activation(dow_f[:, co:co + cw], deg_own_ps[ci][:],
                                     AF.Ln)
            nc.scalar.activation(dow_f[:], dow_f[:], AF.Exp, scale=-0.5)
            down = cpool.tile([P, MT], F32)
            for m in range(MT):
                psd = ppd.tile([P, 1], F32, tag="dacc", name="spread")
                nc.tensor.matmul(psd[:], dow_f[:, m * P:(m + 1) * P], ones_11[:],
                                 start=True, stop=True)
                nc.vector.tensor_copy(out=down[:, m:m + 1], in_=psd[:])

            # xw = d_own * (x_c @ W1 + b1), bf16; AllGather split per group
            cc_xw_in = [dram.tile([gw * P, H], BF, tag="ccxi", name="ccxi",
                                  bufs=len(groups)) for (_, gw) in groups]
            cc_xw_out = [dram.tile([NC * gw * P, H], BF, tag="ccxo", name="ccxo",
                                   bufs=len(groups)) for (_, gw) in groups]
            for gi, (g0, gw) in enumerate(groups):
                xw_ps = [pp.tile([P, H], F32, tag="acc", name="xwacc")
                         for _ in range(gw)]
                for kd in range(DKB):
                    xt = xpool.tile([P, R], BF, tag="xblk", name="xblk")
                    nc.sync.dma_start(out=xt[:], in_=xT[kd * P:(kd + 1) * P, :])
                    for mi in range(gw):
                        m = g0 + mi
                        nc.tensor.matmul(xw_ps[mi][:], xt[:, m * P:(m + 1) * P],
                                         w1_t[:, kd * H:(kd + 1) * H],
                                         start=(kd == 0), stop=(kd == DKB - 1))
                for mi in range(gw):
                    m = g0 + mi
                    xwf = wpool.tile([P, H], F32, tag="xwf", name="xwf")
                    nc.vector.tensor_tensor(out=xwf[:], in0=xw_ps[mi][:],
                                            in1=b1_t[:], op=ADD)
                    xwb = epool.tile([P, H], BF, tag="ev", name="xwb")
                    nc.vector.tensor_scalar(out=xwb[:], in0=xwf[:],
                                            scalar1=down[:, m:m + 1],
                                            scalar2=None, op0=MUL)
                    nc.gpsimd.dma_start(out=cc_xw_in[gi][mi * P:(mi + 1) * P, :],
                                        in_=xwb[:])
                nc.gpsimd.collective_compute(AG, BYP, replica_groups=rg,
                                             ins=[cc_xw_in[gi].opt()],
                                             outs=[cc_xw_out[gi].opt()])

            # ------- phase 2: S1 = (A+I)_cols.T @ YW_full -------
            # gathered k-block for (AG-chunk gi, rank r, sub jb):
            #   global k = r*MT + g0[gi] + jb ; rows (r*gw+jb)*P of cc_xw_out[gi]
            hT = [cpool.tile([P, R], BF, tag="hT", name="hT", bufs=H // P)
                  for _ in range(H // P)]
            cc_m2_in = [dram.tile([gw * P, K3], BF, tag="ccmi", name="ccmi",
                                  bufs=len(groups)) for (_, gw) in groups]
            cc_m2_out = [dram.tile([NC * gw * P, K3], BF, tag="ccmo", name="ccmo",
                                   bufs=len(groups)) for (_, gw) in groups]

            def prop_pass(cc_out_list, width, acc_w, evict_fn):
                """Two grouped accumulation passes over all gathered blocks."""
                for g0a, gwa in groups:          # accumulation m-groups
                    accs = [pp.tile([P, acc_w], F32, tag="acc", name="pacc")
                            for _ in range(gwa)]
                    ki = 0
                    for gi, (g0, gw) in enumerate(groups):   # AG chunks
                        for r in range(NC):
                            gb = gpool.tile([P, gw, width], BF, tag="gblk",
                                            name="gblk")
                            nc.sync.dma_start(
                                out=gb[:],
                                in_=cc_out_list[gi][r * gw * P:(r + 1) * gw * P, :]
                                .rearrange("(k p) c -> p k c", p=P))
                            for jb in range(gw):
                                k = r * MT + g0 + jb
                                for mi in range(gwa):
                                    m = g0a + mi
                                    nc.tensor.matmul(
                                        accs[mi][:], ablk(k)[:, m * P:(m + 1) * P],
                                        gb[:, jb, :], start=(ki == 0),
                                        stop=(ki == NKB - 1))
                                ki += 1
                    for mi in range(gwa):
                        evict_fn(g0a + mi, accs[mi])

            def s1_evict(m, acc):
                gi, off = m // MG, m % MG
                htmp = wpool.tile([P, H], F32, tag="htmp", name="htmp")
                nc.scalar.activation(htmp[:], acc[:], AF.Lrelu,
                                     scale=down[:, m:m + 1], alpha=0.01)
                for hcv in range(H // P):
                    pst = pp.tile([P, P], F32, tag="acc", name="ptr")
                    nc.tensor.transpose(pst[:], htmp[:, hcv * P:(hcv + 1) * P],
                                        identity[:])
                    nc.vector.tensor_copy(out=hT[hcv][:, m * P:(m + 1) * P],
                                          in_=pst[:])
                ps2 = pp.tile([P, K3], F32, tag="acc", name="m2acc")
                for hcv in range(H // P):
                    nc.tensor.matmul(ps2[:], hT[hcv][:, m * P:(m + 1) * P],
                                     wall_t[:, hcv * K3:(hcv + 1) * K3],
                                     start=(hcv == 0), stop=(hcv == H // P - 1))
                m2f = wpool.tile([P, K3], F32, tag="m2f", name="m2f")
                nc.vector.tensor_tensor(out=m2f[:], in0=ps2[:], in1=ball_t[:],
                                        op=ADD)
                m2b = epool.tile([P, K3], BF, tag="ev", name="m2b")
                nc.vector.tensor_scalar(out=m2b[:], in0=m2f[:],
                                        scalar1=down[:, m:m + 1],
                                        scalar2=None, op0=MUL)
                nc.gpsimd.dma_start(out=cc_m2_in[gi][off * P:(off + 1) * P, :],
                                    in_=m2b[:])
                if off == (groups[gi][1] - 1):   # last m of this AG chunk
                    nc.gpsimd.collective_compute(
                        AG, BYP, replica_groups=rg,
                        ins=[cc_m2_in[gi].opt()], outs=[cc_m2_out[gi].opt()])

            prop_pass(cc_xw_out, H, H, s1_evict)

            # ------- phase 3: S2 -------
            s2f8 = cpool.tile([P, MT, K3], F32)

            def s2_evict(m, acc):
                nc.vector.tensor_scalar(out=s2f8[:, m, :], in0=acc[:],
                                        scalar1=down[:, m:m + 1],
                                        scalar2=None, op0=MUL)

            prop_pass(cc_m2_out, K3, K3, s2_evict)

            # ------- batched elementwise chain over all m-tiles -------
            pi_v = s2f8[:, :, 0:K]
            rm_v = s2f8[:, :, K:2 * K]
            rls_v = s2f8[:, :, 2 * K:3 * K]
            ca = cpool.tile([P, MT, K], F32)
            nc.sync.dma_start(out=ca[:],
                              in_=u_beta.ap().rearrange("(m p) t -> p m t", p=P))
            un8 = cpool.tile([P, MT, K], F32)
            nc.sync.dma_start(out=un8[:],
                              in_=u_bern.ap().rearrange("(m p) t -> p m t", p=P))
            er8 = cpool.tile([P, MT, K], F32)
            nc.sync.dma_start(out=er8[:],
                              in_=eps_r.ap().rearrange("(m p) t -> p m t", p=P))
            cb = cpool.tile([P, MT, K], F32)
            cc_t = cpool.tile([P, MT, K], F32)
            # LV = ln((1 - u^(1/b))^(1/a) + EPS)   (in-place in ca)
            nc.scalar.activation(ca[:], ca[:], AF.Ln)
            nc.vector.tensor_tensor(out=ca[:], in0=ca[:], in1=invb8[:], op=MUL)
            nc.scalar.activation(ca[:], ca[:], AF.Exp)
            nc.vector.tensor_scalar(out=ca[:], in0=ca[:], scalar1=-1.0,
                                    scalar2=1.0, op0=MUL, op1=ADD)
            nc.scalar.activation(ca[:], ca[:], AF.Ln)
            nc.vector.tensor_tensor(out=ca[:], in0=ca[:], in1=inva8[:], op=MUL)
            nc.scalar.activation(ca[:], ca[:], AF.Exp)
            nc.scalar.activation(ca[:], ca[:], AF.Ln, bias=epsb[:])
            # cumsum along K: Hillis-Steele ping-pong
            src, dst = ca, cb
            s = 1
            while s < K:
                nc.vector.tensor_tensor(out=dst[:, :, s:K], in0=src[:, :, s:K],
                                        in1=src[:, :, 0:K - s], op=ADD)
                nc.vector.tensor_copy(out=dst[:, :, 0:s], in_=src[:, :, 0:s])
                src, dst = dst, src
                s *= 2
            cs8, ob = src, dst
            # prior = ln(p+EPS) - ln(1+EPS-p), p = exp(cs)
            nc.scalar.activation(ob[:], cs8[:], AF.Exp)
            nc.scalar.activation(cs8[:], ob[:], AF.Ln, bias=epsb[:])
            nc.vector.tensor_scalar(out=ob[:], in0=ob[:], scalar1=-1.0,
                                    scalar2=1.0 + EPS, op0=MUL, op1=ADD)
            nc.scalar.activation(ob[:], ob[:], AF.Ln)
            nc.vector.tensor_tensor(out=cs8[:], in0=cs8[:], in1=ob[:], op=SUB)
            # y = pi + prior + ln(u+EPS) - ln(1+EPS-u); gate = 1/(1+exp(-y))
            nc.vector.tensor_tensor(out=cs8[:], in0=cs8[:], in1=pi_v, op=ADD)
            nc.scalar.activation(ob[:], un8[:], AF.Ln, bias=epsb[:])
            nc.vector.tensor_tensor(out=cs8[:], in0=cs8[:], in1=ob[:], op=ADD)
            nc.vector.tensor_scalar(out=un8[:], in0=un8[:], scalar1=-1.0,
                                    scalar2=1.0 + EPS, op0=MUL, op1=ADD)
            nc.scalar.activation(un8[:], un8[:], AF.Ln)
            nc.vector.tensor_tensor(out=cs8[:], in0=cs8[:], in1=un8[:], op=SUB)
            nc.scalar.activation(cs8[:], cs8[:], AF.Exp, scale=-1.0)
            nc.vector.tensor_scalar(out=cs8[:], in0=cs8[:], scalar1=1.0,
                                    scalar2=None, op0=ADD)
            nc.vector.reciprocal(cs8[:], cs8[:])
            # r = rm + exp(rls)*eps_r;  z = gate * r
            nc.scalar.activation(cc_t[:], rls_v, AF.Exp)
            nc.vector.tensor_tensor(out=cc_t[:], in0=cc_t[:], in1=er8[:], op=MUL)
            nc.vector.tensor_tensor(out=cc_t[:], in0=cc_t[:], in1=rm_v, op=ADD)
            nc.vector.tensor_tensor(out=cc_t[:], in0=cc_t[:], in1=cs8[:], op=MUL)
            z8 = cc_t
            zT = cpool.tile([K, R], BF)
            for m in range(MT):
                psz = pp.tile([P, P], F32, tag="acc", name="ztr")
                nc.tensor.transpose(psz[:K, :], z8[:, m, :], identity[:])
                nc.vector.tensor_copy(out=zT[:, m * P:(m + 1) * P],
                                      in_=psz[:K, :])

            # ------- phase 4: decode -------
            # xhat = z @ Wx + bx  (fills PE while the h2 AllGather runs)
            for m in range(MT):
                for (bo, bw) in _chunks(D, 1024):
                    stx = spool.tile([P, 1024], F32, tag="st", name="stx")
                    for (co, cw) in _chunks(bw, 512):
                        psx = pp.tile([P, 512], F32, tag="acc", name="xhacc")
                        nc.tensor.matmul(psx[:, :cw], zT[:, m * P:(m + 1) * P],
                                         wx_t[:, bo + co:bo + co + cw],
                                         start=True, stop=True)
                        nc.vector.tensor_tensor(out=stx[:, co:co + cw],
                                                in0=psx[:, :cw],
                                                in1=bx_t[:, bo + co:bo + co + cw],
                                                op=ADD)
                    nc.sync.dma_start(out=xhat_out[m * P:(m + 1) * P, bo:bo + bw],
                                      in_=stx[:, :bw])

            h2T = cpool.tile([HD, R], BF)
            cc_h2_in = dram.tile([HD, R], BF)
            for m in range(MT):
                psh = pp.tile([P, HD], F32, tag="acc", name="h2acc")
                nc.tensor.matmul(psh[:], zT[:, m * P:(m + 1) * P], wd_t[:],
                                 start=True, stop=True)
                h2f = wpool.tile([P, HD], F32, tag="h2f", name="h2f")
                nc.vector.tensor_tensor(out=h2f[:], in0=psh[:], in1=bd_t[:],
                                        op=ADD)
                pst2 = pp.tile([P, P], F32, tag="acc", name="h2tr")
                nc.tensor.transpose(pst2[:HD, :], h2f[:], identity[:])
                nc.vector.tensor_copy(out=h2T[:, m * P:(m + 1) * P],
                                      in_=pst2[:HD, :])
            nc.gpsimd.dma_start(out=cc_h2_in[:], in_=h2T[:])
            cc_h2_out = dram.tile([NC * HD, R], BF)
            nc.gpsimd.collective_compute(AG, BYP, replica_groups=rg,
                                         ins=[cc_h2_in.opt()],
                                         outs=[cc_h2_out.opt()])

            # edge = h2 @ h2_full.T  (batched 512KB stores)
            for nr in range(NC):
                rblk = gpool.tile([HD, R], BF, tag="rblk", name="rblk")
                nc.sync.dma_start(out=rblk[:],
                                  in_=cc_h2_out[nr * HD:(nr + 1) * HD, :])
                for m in range(MT):
                    ste = spool.tile([P, 1024], F32, tag="st", name="ste")
                    for ci, (co, cw) in enumerate(_chunks(R, 512)):
                        pse = pp.tile([P, 512], F32, tag="acc", name="edacc")
                        nc.tensor.matmul(pse[:, :cw], h2T[:, m * P:(m + 1) * P],
                                         rblk[:, co:co + cw],
                                         start=True, stop=True)
                        if (m + ci) % 2 == 0:
                            nc.vector.tensor_copy(out=ste[:, co:co + cw],
                                                  in_=pse[:, :cw])
                        else:
                            nc.scalar.copy(ste[:, co:co + cw], pse[:, :cw])
                    nc.sync.dma_start(
                        out=edge_out[m * P:(m + 1) * P, nr * R:nr * R + R],
                        in_=ste[:, :R])

    nc.compile()
    return nc


def make_in_maps(cfg: Cfg, inputs):
    N, D, H, K, HD, NC, R = cfg.N, cfg.D, cfg.H, cfg.K, cfg.HD, cfg.NC, cfg.R
    bf = ml_dtypes.bfloat16
    f32 = np.float32
    A = np.asarray(inputs['adj_mat'], f32)
    x = np.asarray(inputs['x'], f32)
    Abf = A.astype(bf)
    xTbf = np.ascontiguousarray(np.asarray(x.T)).astype(bf)
    shared = {
        'w1': np.asarray(inputs['W1'], f32).astype(bf),
        'b1': np.asarray(inputs['b1'], f32).reshape(1, H),
        'wall': np.concatenate([np.asarray(inputs['Wpi'], f32),
                                np.asarray(inputs['Wm'], f32),
                                np.asarray(inputs['Wls'], f32)],
                               axis=1).astype(bf),
        'ball': np.concatenate([np.asarray(inputs['bpi'], f32),
                                np.asarray(inputs['bm'], f32),
                                np.asarray(inputs['bls'], f32)]).reshape(1, 3 * K),
        'wd': np.asarray(inputs['Wd'], f32).astype(bf),
        'bd': np.asarray(inputs['bd'], f32).reshape(1, HD),
        'wx': np.asarray(inputs['Wx'], f32).astype(bf),
        'bx': np.asarray(inputs['bx'], f32).reshape(1, D),
        'beta_a': np.asarray(inputs['beta_a'], f32).reshape(1, K),
        'beta_b': np.asarray(inputs['beta_b'], f32).reshape(1, K),
        'tri': np.triu(np.ones((K, K), f32)),
        'ident': np.eye(P, dtype=f32),
    }
    u_beta = np.asarray(inputs['u_beta'], f32)
    u_bern = np.asarray(inputs['u_bern'], f32)
    eps_r = np.asarray(inputs['eps_r'], f32)
    in_maps = []
    diag = np.arange(R)
    NKB = N // P
    for c in range(NC):
        c0 = c * R
        ac = np.ascontiguousarray(Abf[:, c0:c0 + R])
        ac[c0 + diag, diag] += np.asarray(1.0, bf)   # fold in +I (exact in bf16)
        # swizzle to partition-major: row p holds [k0|k1|...] col-slices
        ac = np.ascontiguousarray(
            ac.reshape(NKB, P, R).transpose(1, 0, 2).reshape(P, NKB * R))
        in_maps.append({
            'a_cols': ac,
            'xT': np.ascontiguousarray(xTbf[:, c0:c0 + R]),
            'u_beta': np.ascontiguousarray(u_beta[c0:c0 + R]),
            'u_bern': np.ascontiguousarray(u_bern[c0:c0 + R]),
            'eps_r': np.ascontiguousarray(eps_r[c0:c0 + R]),
            **shared,
        })
    return in_maps


_CACHE = {}


def _get_nc(cfg: Cfg):
    key = (cfg.N, cfg.D, cfg.H, cfg.K, cfg.HD, cfg.NC)
    if key not in _CACHE:
        _CACHE[key] = build_bass(cfg)
    return _CACHE[key]


def run(cfg: Cfg, inputs, trace=False):
    nc = _get_nc(cfg)
    in_maps = make_in_maps(cfg, inputs)
    res = run_bass_kernel_spmd(nc, in_maps, list(range(cfg.NC)), trace=trace)
    xhat = np.concatenate([r['xhat'] for r in res.results], axis=0)
    edge = np.concatenate([r['edge'] for r in res.results], axis=0)
    return (xhat.reshape(-1).astype(np.float32),
            edge.reshape(-1).astype(np.float32)), res


def kernel(**inputs):
    cfg = Cfg()
    out, _ = run(cfg, inputs, trace=False)
    return out
